# revision 1
# baseline (speedup 1.0000x reference)
"""GAT (2 layers: 4 heads x 32, then 1 head x 40) on 8 TRN2 NeuronCores.

Sharding: dst-node sharding across 8 cores. Per core, its NPC nodes are
host-packed into `n_blocks` blocks of <=128 nodes, balanced so that every
block's incoming-edge count (split into lo/hi src streams for int16
dma_gather addressing) fits a FIXED number of 128-edge chunks (C_lo / C_hi)
-- this makes the SPMD program structurally identical across cores while all
per-core irregularity lives in the input data (gather indices, dst-slot ids,
permuted x slices).

Per-edge pipeline (one 128-edge chunk):
  dma_gather slab rows [h1 interleaved with 1.0 cols | a_s] by src;
  D[e,d] = onehot(dst_slot) via DVE tensor_scalar is_equal vs iota;
  D_T via PE transpose; a_d per edge = D_T.T @ a_d_block (PE);
  w = exp(leaky_relu(a_s + a_d)) (DVE add, ACT lrelu, ACT exp+broadcast);
  Msg = G * w_exp (DVE); psum_blk += D.T @ Msg (PE) -- the interleaved 1.0
  columns turn into the softmax normalizers z_h.
Block epilogue: h1 = psum/z, h2 = elu(h1), transpose, @ [W2|ws2|wd2] -> L2
table row [h2p(40) | 1.0 | a_s2] + resident a_d2. AllGather L2 table, run the
same edge pipeline for layer 2, out = psum2/z2.
"""
import sys, os, types, time
sys.path.insert(0, '/opt/trn_rl_repo')
import numpy as np
import ml_dtypes

import concourse.bass as bass
import concourse.bacc as bacc
import concourse.mybir as mybir
import concourse.tile as tile
from concourse.vector_clock import ScopedClock
from concourse.masks import make_identity

BF16 = mybir.dt.bfloat16
F32 = mybir.dt.float32
AF = mybir.ActivationFunctionType
ALU = mybir.AluOpType
NPBF = ml_dtypes.bfloat16

# ---------------------------------------------------------------- compat ---
_MAXW = 1

def _patched_drain_and_barrier(self, tick_clock, wait_clock):
    nc = self.nc
    drain_inst = nc.sync.drain()
    wait_clock.add_sem_waits(drain_inst.ins, ScopedClock({None: tick_clock.global_clock}))
    si = drain_inst.ins.sync_info
    waits = list(si.on_wait or [])
    if len(waits) > _MAXW:
        si.on_wait.clear()
        si.on_wait.extend(waits[:_MAXW])
        rest = waits[_MAXW:]
        for i in range(0, len(rest), _MAXW):
            extra = nc.sync.drain()
            esi = extra.ins.sync_info
            if esi is None:
                extra.ins.sync_info = mybir.SyncInfo(on_wait=[], on_update=[])
                esi = extra.ins.sync_info
            esi.on_wait.extend(rest[i:i + _MAXW])
    nc.all_engine_barrier()
    assert self.sems is not None
    popped = nc._tile_sem_poison_stack.pop()
    assert popped is self._sem_poison
    nc.clear_and_free_semaphores(list(self.sems.allocated().values()))
    nc.all_engine_barrier()

tile.TileContext._drain_and_barrier = _patched_drain_and_barrier


def legalize_waits(nc, maxw=_MAXW):
    n_split = 0
    for fn in nc.m.functions:
        for bb in fn.blocks:
            out = []
            for ins in bb.instructions:
                si = ins.sync_info
                waits = list(si.on_wait) if (si is not None and si.on_wait) else []
                if len(waits) > maxw:
                    extras, keep = waits[:-maxw], waits[-maxw:]
                    for j, w in enumerate(extras):
                        nop = mybir.InstEventSemaphore(name=f"{ins.name}_xw{j}", ins=[], outs=[])
                        nop.engine = ins.engine
                        nop.sync_info = mybir.SyncInfo(on_wait=[w], on_update=[])
                        out.append(nop)
                        n_split += 1
                    si.on_wait.clear()
                    si.on_wait.extend(keep)
                out.append(ins)
            bb.instructions[:] = out
    return n_split


def install_ntff_hook():
    if "antenv.axon_hooks" in sys.modules:
        return
    m = types.ModuleType("antenv.axon_hooks")
    _hook = [None]
    m.set_axon_ntff_profile_hook = lambda h: _hook.__setitem__(0, h)
    m.get_axon_ntff_profile_hook = lambda: _hook[0]
    sys.modules["antenv.axon_hooks"] = m
    import antenv
    antenv.axon_hooks = m
    try:
        from trn_agent_boot.trn_boot import _ntff_profile_via_ctypes
        m.set_axon_ntff_profile_hook(_ntff_profile_via_ctypes('/opt/axon/libaxon_pjrt.so'))
    except Exception:
        pass


# ---------------------------------------------------------------- config ---
class Cfg:
    def __init__(self, N=50000, F_IN=128, HID=32, HEADS=4, N_CLS=40, E=800000,
                 n_cores=8, slab_chunks=24, neg_slope=0.2, split=32768):
        self.N, self.F_IN, self.HID, self.HEADS, self.N_CLS, self.E = N, F_IN, HID, HEADS, N_CLS, E
        self.n_cores = n_cores
        self.SC = slab_chunks                 # chunks per gather slab
        self.neg_slope = neg_slope
        assert N % n_cores == 0
        self.NPC = N // n_cores
        self.n_blocks = (self.NPC + 127) // 128
        self.NPCP = self.n_blocks * 128       # padded nodes per core
        self.HO = HEADS * HID                 # 128
        self.L1C = self.HO + HEADS            # 132 msg cols (features + z cols)
        self.L1W = self.L1C + HEADS           # 136 written cols (plus a_s)
        self.L1ROW = 256                      # bf16 elems per table row (512B)
        self.L2C = N_CLS + 1                  # 41 msg cols (h2p + z col)
        self.L2W = N_CLS + 2                  # 42 written cols (plus a_s2)
        self.L2ROW = 128                      # bf16 per row (256B)
        self.SPLIT = split
        assert split % 128 == 0
        self.NP = ((N + 127) // 128) * 128    # padded node count (dense tiles)
        self.NG = self.n_cores * self.NPCP    # global permuted L2 table rows
        # filled by prep: C_lo, C_hi


# ------------------------------------------------------------ host prep ---
def _pack_blocks(cfg, deg_lo, deg_hi):
    """Greedy balance nodes (local ids 0..NPC-1) into n_blocks blocks of
    <=128 nodes minimizing the max lo/hi edge load. Returns perm [n_blocks,128]
    (local node id or -1)."""
    nb = cfg.n_blocks
    order = np.argsort(-(deg_lo + deg_hi), kind='stable')
    cnt = np.zeros(nb, np.int64)
    lo = np.zeros(nb, np.int64)
    hi = np.zeros(nb, np.int64)
    perm = -np.ones((nb, 128), np.int64)
    for v in order:
        w = deg_lo[v] + deg_hi[v]
        # pick feasible block with min total load
        feas = np.where(cnt < 128)[0]
        b = feas[np.argmin(lo[feas] + hi[feas])]
        perm[b, cnt[b]] = v
        cnt[b] += 1
        lo[b] += deg_lo[v]
        hi[b] += deg_hi[v]
    return perm, lo, hi


def prep_core(cfg, src, dst, core):
    """Per-core packing. Returns dict with perm, per-(block,stream) edge
    arrays (src-sorted), and counts."""
    base = core * cfg.NPC
    m = (dst >= base) & (dst < base + cfg.NPC)
    s, d = src[m], dst[m] - base
    deg_lo = np.bincount(d[s < cfg.SPLIT], minlength=cfg.NPC)
    deg_hi = np.bincount(d[s >= cfg.SPLIT], minlength=cfg.NPC)
    perm, lo_sum, hi_sum = _pack_blocks(cfg, deg_lo, deg_hi)
    # node -> (block, slot)
    blk_of = -np.ones(cfg.NPC, np.int64)
    slot_of = -np.ones(cfg.NPC, np.int64)
    for b in range(cfg.n_blocks):
        for j in range(128):
            v = perm[b, j]
            if v >= 0:
                blk_of[v] = b
                slot_of[v] = j
    eb = blk_of[d]
    es = slot_of[d]
    per = {}
    for b in range(cfg.n_blocks):
        mb = eb == b
        sb, slb = s[mb], es[mb]
        for st in (0, 1):
            ms = (sb < cfg.SPLIT) if st == 0 else (sb >= cfg.SPLIT)
            ss, sl = sb[ms], slb[ms]
            o = np.argsort(ss, kind='stable')
            per[(b, st)] = (ss[o], sl[o])
    return {"perm": perm, "per": per,
            "c_lo": int(np.max(np.ceil(lo_sum / 128))) if len(lo_sum) else 0,
            "c_hi": int(np.max(np.ceil(hi_sum / 128))) if len(hi_sum) else 0}


def build_streams(cfg, pc, l2_bases):
    """Given per-core packing pc and fixed C_lo/C_hi, build the gather idx
    arrays and dst-slot arrays for L1 (raw node ids) and L2 (permuted global
    ids via l2_bases[node] = permuted row)."""
    C = {0: cfg.C_lo, 1: cfg.C_hi}
    out = {}
    for layer, xl in (("l1", None), ("l2", l2_bases)):
        for st in (0, 1):
            nchunks = cfg.n_blocks * C[st]
            idx = np.zeros((nchunks, 128), np.int64)
            dl = np.full((nchunks, 128), 255, np.int64)
            ci = 0
            for b in range(cfg.n_blocks):
                ss, sl = pc["per"][(b, st)]
                if layer == "l2":
                    g = xl[ss]                       # permuted global rows
                    g = np.sort(g)                    # locality; slot must follow!
                    # NOTE: need slots aligned with sorted order:
                    o = np.argsort(xl[ss], kind='stable')
                    g, sl2 = xl[ss][o], sl[o]
                    base_off = cfg.SPLIT if st == 1 else 0
                    # re-split: permuted ids may cross SPLIT differently!
                    # handled by caller building separate l2 streams.
                    v = g - base_off
                    svals, slvals = v, sl2
                else:
                    svals = ss - (cfg.SPLIT if st == 1 else 0)
                    slvals = sl
                n = len(svals)
                cap = C[st] * 128
                assert n <= cap, (n, cap)
                a = np.zeros(cap, np.int64)
                a[:n] = svals
                dd = np.full(cap, 255, np.int64)
                dd[:n] = slvals
                for c in range(C[st]):
                    idx[ci] = a[c * 128:(c + 1) * 128]
                    dl[ci] = dd[c * 128:(c + 1) * 128]
                    ci += 1
            out[(layer, st)] = (idx, dl)
    return out


def prep_host(cfg, inputs):
    x = np.asarray(inputs["x"], np.float32)
    W1 = np.asarray(inputs["W1"], np.float32)
    as1 = np.asarray(inputs["att_src1"], np.float32)
    ad1 = np.asarray(inputs["att_dst1"], np.float32)
    W2 = np.asarray(inputs["W2"], np.float32)
    as2 = np.asarray(inputs["att_src2"], np.float32)
    ad2 = np.asarray(inputs["att_dst2"], np.float32)
    ei = np.asarray(inputs["edge_index"])
    loops = np.arange(cfg.N, dtype=np.int64)
    src = np.concatenate([ei[0].astype(np.int64), loops])
    dst = np.concatenate([ei[1].astype(np.int64), loops])

    H, HID = cfg.HEADS, cfg.HID
    # W1il: interleaved feature cols with zero cols at the z positions
    W1r = W1.reshape(cfg.F_IN, H, HID)
    W1il = np.zeros((cfg.F_IN, cfg.L1C), np.float32)
    for h in range(H):
        W1il[:, 33 * h:33 * h + HID] = W1r[:, h]
    Ws1 = (W1r * as1[None]).sum(-1)
    Wd1 = (W1r * ad1[None]).sum(-1)
    W1ext = np.concatenate([W1il, Ws1, Wd1], 1).astype(NPBF)   # [128, 140]
    ws2 = (W2 * as2).sum(1, keepdims=True)
    wd2 = (W2 * ad2).sum(1, keepdims=True)
    W2ext = np.concatenate([W2, ws2, wd2], 1).astype(NPBF)     # [128, 42]

    xT = np.zeros((cfg.F_IN, cfg.NP), NPBF)
    xT[:, :cfg.N] = np.ascontiguousarray(x.T).astype(NPBF)
    iota = np.broadcast_to(np.arange(128, dtype=np.float32), (128, 128)).astype(NPBF)

    # pass 1: per-core packing, find global C_lo/C_hi
    pcs = [prep_core(cfg, src, dst, k) for k in range(cfg.n_cores)]
    cfg.C_lo = max(p["c_lo"] for p in pcs)
    cfg.C_hi = max(p["c_hi"] for p in pcs)

    # L2 permuted global row of node v
    l2_bases = np.zeros(cfg.N, np.int64)
    for k in range(cfg.n_cores):
        perm = pcs[k]["perm"]
        for b in range(cfg.n_blocks):
            for j in range(128):
                v = perm[b, j]
                if v >= 0:
                    l2_bases[k * cfg.NPC + v] = k * cfg.NPCP + b * 128 + j

    # L2 streams must be re-split by PERMUTED id (lo: <SPLIT, hi: >=SPLIT).
    # Simplest correct approach: recompute per-core per-block streams in the
    # permuted-id space with its own balance constraint... but the block
    # structure (C_lo/C_hi chunks) is shared with L1. Instead: keep the SAME
    # chunk structure (same edge->chunk assignment as L1) and allow l2 idx
    # to go out of int16 range? No. Resolution: build L2 edge streams
    # independently (re-sorted by permuted src id and re-split lo/hi), with
    # their own fixed chunk counts C2_lo/C2_hi.
    per2 = []
    c2lo = c2hi = 0
    for k in range(cfg.n_cores):
        base = k * cfg.NPC
        m = (dst >= base) & (dst < base + cfg.NPC)
        s0, d0 = src[m], dst[m] - base
        perm = pcs[k]["perm"]
        blk_of = -np.ones(cfg.NPC, np.int64)
        slot_of = -np.ones(cfg.NPC, np.int64)
        for b in range(cfg.n_blocks):
            for j in range(128):
                v = perm[b, j]
                if v >= 0:
                    blk_of[v] = b
                    slot_of[v] = j
        g = l2_bases[s0]                       # permuted global src row
        eb, es = blk_of[d0], slot_of[d0]
        per = {}
        for b in range(cfg.n_blocks):
            mb = eb == b
            gb, sb = g[mb], es[mb]
            for st in (0, 1):
                ms = (gb < cfg.SPLIT) if st == 0 else (gb >= cfg.SPLIT)
                gg, sl = gb[ms], sb[ms]
                o = np.argsort(gg, kind='stable')
                per[(b, st)] = (gg[o], sl[o])
                n = len(gg)
                if st == 0:
                    c2lo = max(c2lo, int(np.ceil(n / 128)))
                else:
                    c2hi = max(c2hi, int(np.ceil(n / 128)))
        per2.append(per)
    cfg.C2_lo, cfg.C2_hi = c2lo, c2hi

    def pack_stream(per, b_range, st, C, base_off):
        nchunks = cfg.n_blocks * C
        idx = np.zeros((nchunks, 128), np.int64)
        dl = np.full((nchunks, 128), 255, np.int64)
        ci = 0
        for b in b_range:
            ss, sl = per[(b, st)]
            n = len(ss)
            cap = C * 128
            assert n <= cap
            a = np.zeros(cap, np.int64)
            a[:n] = ss - base_off
            dd = np.full(cap, 255, np.int64)
            dd[:n] = sl
            for c in range(C):
                idx[ci] = a[c * 128:(c + 1) * 128]
                dl[ci] = dd[c * 128:(c + 1) * 128]
                ci += 1
        return idx, dl

    def to_gather_layout(cfg, idx_chunks, dl_chunks, SC):
        """idx_chunks [nch, 128] -> slabs of SC chunks:
        idx16 [nslab, 128, SC*8] (i at partition i%16, free i//16, 8x stripes)
        dl bf16 [nslab, 128, SC]."""
        nch = idx_chunks.shape[0]
        nslab = (nch + SC - 1) // SC
        idx16 = np.zeros((nslab, 128, SC * 8), np.int16)
        dlb = np.full((nslab, 128, SC), 255.0, np.float32)
        sizes = []
        for si in range(nslab):
            c0, c1 = si * SC, min((si + 1) * SC, nch)
            n = (c1 - c0) * 128
            flat = idx_chunks[c0:c1].reshape(-1)   # chunk-major, i = c*128+p
            w = flat.astype(np.int16).reshape(-1, 16).T  # [16, n/16]
            idx16[si, :, :w.shape[1]] = np.tile(w, (8, 1))
            dlb[si, :, :c1 - c0] = dl_chunks[c0:c1].T.astype(np.float32)
            sizes.append(n)
        return idx16, dlb, sizes

    per_core, metas = [], []
    for k in range(cfg.n_cores):
        pc, p2 = pcs[k], per2[k]
        im = {"xT": np.asarray(xT), "W1ext": np.asarray(W1ext),
              "W2ext": np.asarray(W2ext), "iota": np.asarray(iota)}
        # permuted own x slice (padded slots -> zero cols)
        xo = np.zeros((cfg.F_IN, cfg.NPCP), NPBF)
        perm = pc["perm"]
        for b in range(cfg.n_blocks):
            for j in range(128):
                v = perm[b, j]
                if v >= 0:
                    xo[:, b * 128 + j] = xT[:, k * cfg.NPC + v]
        im["xT_own"] = xo
        meta = {"perm": perm}
        for tag, per, Clo, Chi, split in (
                ("a", pc["per"], cfg.C_lo, cfg.C_hi, cfg.SPLIT),
                ("b", p2, cfg.C2_lo, cfg.C2_hi, cfg.SPLIT)):
            for st, C in ((0, Clo), (1, Chi)):
                idxc, dlc = pack_stream(per, range(cfg.n_blocks), st, C, split if st else 0)
                i16, dlb, sizes = to_gather_layout(cfg, idxc, dlc, cfg.SC)
                im[f"idx_{tag}{st}"] = i16
                im[f"dl_{tag}{st}"] = np.asarray(dlb)
                meta[f"sizes_{tag}{st}"] = sizes
        per_core.append(im)
        metas.append(meta)
    shared = {"metas": metas, "l2_bases": l2_bases, "pcs": pcs}
    return per_core, shared


# ------------------------------------------------------------ device build ---
def build_kernel(cfg, shared, legalize=True):
    SC = cfg.SC
    sizes_a0 = shared["metas"][0]["sizes_a0"]
    sizes_a1 = shared["metas"][0]["sizes_a1"]
    sizes_b0 = shared["metas"][0]["sizes_b0"]
    sizes_b1 = shared["metas"][0]["sizes_b1"]
    ns_a0, ns_a1 = len(sizes_a0), len(sizes_a1)
    ns_b0, ns_b1 = len(sizes_b0), len(sizes_b1)
    NT = cfg.NP // 128
    H = cfg.HEADS

    nc = bacc.Bacc("TRN2", target_bir_lowering=False, num_devices=cfg.n_cores)

    xT = nc.dram_tensor("xT", [cfg.F_IN, cfg.NP], BF16, kind="ExternalInput")
    xT_own = nc.dram_tensor("xT_own", [cfg.F_IN, cfg.NPCP], BF16, kind="ExternalInput")
    W1ext = nc.dram_tensor("W1ext", [cfg.F_IN, cfg.L1C + 2 * H], BF16, kind="ExternalInput")
    W2ext = nc.dram_tensor("W2ext", [cfg.HO, cfg.N_CLS + 2], BF16, kind="ExternalInput")
    iota_in = nc.dram_tensor("iota", [128, 128], BF16, kind="ExternalInput")
    gin = {}
    for tag, ns in (("a0", ns_a0), ("a1", ns_a1), ("b0", ns_b0), ("b1", ns_b1)):
        gin[f"idx_{tag}"] = nc.dram_tensor(f"idx_{tag}", [max(ns, 1), 128, SC * 8], mybir.dt.int16, kind="ExternalInput")
        gin[f"dl_{tag}"] = nc.dram_tensor(f"dl_{tag}", [max(ns, 1), 128, SC], F32, kind="ExternalInput")
    out = nc.dram_tensor("out", [cfg.NPCP, cfg.N_CLS], F32, kind="ExternalOutput")

    l1lo = nc.dram_tensor("l1lo", [cfg.SPLIT, cfg.L1ROW], BF16)
    l1hi = nc.dram_tensor("l1hi", [cfg.NP - cfg.SPLIT, cfg.L1ROW], BF16)
    l2own = nc.dram_tensor("l2own", [cfg.NPCP, cfg.L2ROW], BF16)
    l2tab = nc.dram_tensor("l2tab", [cfg.NG, cfg.L2ROW], BF16, addr_space="Shared")
    l2hi = nc.dram_tensor("l2hi", [cfg.NG - cfg.SPLIT, cfg.L2ROW], BF16)

    with tile.TileContext(nc) as tc:
        regcache = {}
        def reg(v):
            if v not in regcache:
                regcache[v] = nc.gpsimd.to_reg(v)
            return regcache[v]

        with tc.tile_pool(name="consts", bufs=1) as consts:
            iota_t = consts.tile([128, 128], BF16)
            nc.sync.dma_start(iota_t[:], iota_in[:])
            ident = consts.tile([128, 128], BF16)
            make_identity(nc, ident[:])
            w1_t = consts.tile([cfg.F_IN, cfg.L1C + 2 * H], BF16)
            nc.sync.dma_start(w1_t[:], W1ext[:])
            w2_t = consts.tile([cfg.HO, cfg.N_CLS + 2], BF16)
            nc.sync.dma_start(w2_t[:], W2ext[:])
            adall = consts.tile([128, cfg.n_blocks * H], BF16)
            ad2all = consts.tile([128, cfg.n_blocks], BF16)

            # ---------------- P0: dense L1 table over all nodes ----------
            with (
                tc.tile_pool(name="p0", bufs=4) as p0pool,
                tc.tile_pool(name="p0ps", bufs=2, space="PSUM") as p0ps,
            ):
                for t2 in range((NT + 1) // 2):
                    nh = 2 if 2 * t2 + 1 < NT else 1
                    xt = p0pool.tile([cfg.F_IN, 256], BF16, tag="xt")
                    nc.sync.dma_start(xt[:, 0:nh * 128], xT[:, t2 * 256:t2 * 256 + nh * 128])
                    for half in range(nh):
                        t = 2 * t2 + half
                        ps = p0ps.tile([128, cfg.L1C + 2 * H], F32, tag="p0ps")
                        nc.tensor.matmul(ps[:], lhsT=xt[:, half * 128:(half + 1) * 128],
                                         rhs=w1_t[:], start=True, stop=True)
                        ev = p0pool.tile([128, cfg.L1W], BF16, tag="ev")
                        nc.scalar.activation(ev[:, 0:cfg.L1C], ps[:, 0:cfg.L1C], AF.Copy)
                        nc.vector.memset(ev[:, 32:cfg.L1C:33], 1.0)
                        nc.scalar.activation(ev[:, cfg.L1C:cfg.L1W], ps[:, cfg.L1C:cfg.L1C + H], AF.Copy)
                        tl = cfg.SPLIT // 128
                        weng = nc.scalar if (t % 2) else nc.sync
                        if t < tl:
                            weng.dma_start(l1lo[t * 128:(t + 1) * 128, 0:cfg.L1W], ev[:])
                        else:
                            weng.dma_start(l1hi[(t - tl) * 128:(t - tl + 1) * 128, 0:cfg.L1W], ev[:])
                # own-range a_d (permuted block layout)
                for b in range(cfg.n_blocks):
                    xo = p0pool.tile([cfg.F_IN, 128], BF16, tag="xt")
                    nc.sync.dma_start(xo[:], xT_own[:, b * 128:(b + 1) * 128])
                    ps = p0ps.tile([128, cfg.L1C + 2 * H], F32, tag="p0ps")
                    nc.tensor.matmul(ps[:, 0:H], lhsT=xo[:], rhs=w1_t[:, cfg.L1C + H:cfg.L1C + 2 * H], start=True, stop=True)
                    nc.scalar.activation(adall[:, H * b:H * b + H], ps[:, 0:H], AF.Copy)

            # ---------------- edge-layer template -------------------------
            def edge_layer(layer):
                if layer == 1:
                    tab_lo, tab_hi, tag_lo, tag_hi = l1lo, l1hi, "a0", "a1"
                    C_lo, C_hi = cfg.C_lo, cfg.C_hi
                    sizes = {0: sizes_a0, 1: sizes_a1}
                    ROW, MC = cfg.L1ROW, cfg.L1C
                else:
                    tab_lo, tab_hi, tag_lo, tag_hi = l2tab, l2hi, "b0", "b1"
                    C_lo, C_hi = cfg.C2_lo, cfg.C2_hi
                    sizes = {0: sizes_b0, 1: sizes_b1}
                    ROW, MC = cfg.L2ROW, cfg.L2C
                NH = H if layer == 1 else 1

                with (
                    tc.tile_pool(name=f"gsl{layer}", bufs=3) as gpool_lo,
                    tc.tile_pool(name=f"gsh{layer}", bufs=3) as gpool_hi,
                    tc.tile_pool(name=f"dl{layer}", bufs=3) as dlpool,
                    tc.tile_pool(name=f"ix{layer}", bufs=3) as ixpool,
                    tc.tile_pool(name=f"wk{layer}", bufs=6) as wk,
                    tc.tile_pool(name=f"dp{layer}", bufs=14) as dp,
                    tc.tile_pool(name=f"dtp{layer}", bufs=14) as dtp,
                    tc.tile_pool(name=f"sc{layer}", bufs=4) as scp,
                    tc.tile_pool(name=f"ps{layer}", bufs=2, space="PSUM") as psacc,
                    tc.tile_pool(name=f"pst{layer}", bufs=2, space="PSUM") as pstr,
                    tc.tile_pool(name=f"psx{layer}", bufs=2, space="PSUM") as psx,
                ):
                    cur = {0: -1, 1: -1}
                    slabs = {0: None, 1: None}
                    dls = {0: None, 1: None}

                    def ensure_slab(st, si):
                        if cur[st] == si:
                            return
                        cur[st] = si
                        pool = gpool_lo if st == 0 else gpool_hi
                        tname = tag_lo if st == 0 else tag_hi
                        it = ixpool.tile([128, SC * 8], mybir.dt.int16, tag=f"it{st}")
                        nc.sync.dma_start(it[:], gin[f"idx_{tname}"][si])
                        dlt = dlpool.tile([128, SC], F32, tag=f"dl{st}")
                        nc.sync.dma_start(dlt[:], gin[f"dl_{tname}"][si])
                        g = pool.tile([128, SC, ROW], BF16, tag=f"g{st}")
                        n = sizes[st][si]
                        tabx = tab_lo if st == 0 else tab_hi
                        nc.gpsimd.dma_gather(
                            out_ap=g[:, 0:max(n // 128, 1), :],
                            in_ap=tabx[:, :],
                            idxs_ap=it[:, 0:max(n // 16, 1)],
                            num_idxs=n, num_idxs_reg=reg(n),
                            elem_size=ROW, single_packet=False)
                        slabs[st] = g
                        dls[st] = dlt

                    for b in range(cfg.n_blocks):
                        pblk = psacc.tile([128, MC], F32, tag="acc")
                        first = True
                        # chunk list: (stream, global chunk id)
                        chs = [(0, b * C_lo + c) for c in range(C_lo)] + \
                              [(1, b * C_hi + c) for c in range(C_hi)]
                        # group for score batching: consecutive chunks in same slab
                        groups = []
                        gcur = None
                        for st, gc in chs:
                            si, col = gc // SC, gc % SC
                            if gcur and gcur[0] == st and gcur[1] == si and len(gcur[2]) < 12:
                                gcur[2].append(col)
                            else:
                                gcur = [st, si, [col]]
                                groups.append(gcur)
                        nch = len(chs)
                        ci_total = 0
                        for st, si, cols in groups:
                            ensure_slab(st, si)
                            g, dlt = slabs[st], dls[st]
                            ng = len(cols)
                            Ds, DTs = [], []
                            adx = psx.tile([128, 12 * NH], F32, tag="adx")
                            for j, col in enumerate(cols):
                                D = dp.tile([128, 128], BF16, tag="D")
                                nc.vector.tensor_scalar(
                                    D[:], iota_t[:], dlt[:, col:col + 1], None,
                                    ALU.is_equal)
                                tp = pstr.tile([128, 128], BF16, tag="tp")
                                nc.tensor.transpose(tp[:], D[:], ident[:])
                                DT = dtp.tile([128, 128], BF16, tag="DT")
                                nc.scalar.activation(DT[:], tp[:], AF.Copy)
                                nc.tensor.matmul(
                                    adx[:, j * NH:(j + 1) * NH], lhsT=DT[:],
                                    rhs=(adall[:, H * b:H * b + H] if layer == 1
                                         else ad2all[:, b:b + 1]),
                                    start=True, stop=True)
                                Ds.append((D, col))
                            # batched scores for the group
                            sc_t = scp.tile([128, 12 * NH], F32, tag="sc")
                            if layer == 1:
                                asrc = g[:, cols[0]:cols[0] + ng, cfg.L1C:cfg.L1C + H]
                            else:
                                asrc = g[:, cols[0]:cols[0] + ng, cfg.N_CLS + 1:cfg.N_CLS + 2]
                            nc.vector.tensor_tensor(
                                out=sc_t[:, 0:ng * NH], in0=adx[:, 0:ng * NH],
                                in1=asrc, op=ALU.add)
                            
                            nc.vector.scalar_tensor_tensor(
                                out=sc_t[:, 0:ng * NH], in0=sc_t[:, 0:ng * NH],
                                scalar=cfg.neg_slope, in1=sc_t[:, 0:ng * NH],
                                op0=ALU.mult, op1=ALU.max)
                            if layer == 1:
                                wex = wk.tile([128, 12, cfg.L1C], BF16, tag="wex")
                                for j in range(ng):
                                    nc.scalar.activation(
                                        wex[:, j, :].rearrange("p (h q) -> p h q", h=H),
                                        sc_t[:, j * H:(j + 1) * H].to_broadcast([128, H, 33]),
                                        AF.Exp)
                            else:
                                w2s = scp.tile([128, 12], F32, tag="w2s")
                                nc.scalar.activation(w2s[:, 0:ng], sc_t[:, 0:ng], AF.Exp)
                            for j, (D, col) in enumerate(Ds):
                                msg = wk.tile([128, MC], BF16, tag="msg")
                                if layer == 1:
                                    nc.vector.tensor_tensor(
                                        out=msg[:], in0=g[:, col, 0:MC],
                                        in1=wex[:, j, :], op=ALU.mult)
                                else:
                                    nc.vector.tensor_scalar(
                                        msg[:], g[:, col, 0:MC], w2s[:, j:j + 1], None,
                                        ALU.mult)
                                ci_total += 1
                                nc.tensor.matmul(pblk[:], lhsT=D[:], rhs=msg[:],
                                                 start=first, stop=(ci_total == nch))
                                first = False
                        # -------- block epilogue --------
                        if layer == 1:
                            zt = scp.tile([128, H], F32, tag="zt")
                            nc.vector.tensor_scalar_max(zt[:], pblk[:, 32:cfg.L1C:33], 1e-30)
                            rz = scp.tile([128, H], F32, tag="rz")
                            nc.vector.reciprocal(rz[:], zt[:])
                            h1f = wk.tile([128, cfg.HO], F32, tag="h1f")
                            for h in range(H):
                                nc.scalar.activation(
                                    h1f[:, 32 * h:32 * h + 32],
                                    pblk[:, 33 * h:33 * h + 32],
                                    AF.Copy, scale=rz[:, h:h + 1])
                            mn = wk.tile([128, cfg.HO], F32, tag="mn")
                            nc.vector.tensor_scalar_min(mn[:], h1f[:], 0.0)
                            ex = wk.tile([128, cfg.HO], F32, tag="ex")
                            nc.scalar.activation(ex[:], mn[:], AF.Exp)
                            rl = wk.tile([128, cfg.HO], F32, tag="rl")
                            nc.vector.tensor_scalar_max(rl[:], h1f[:], 0.0)
                            h2b = wk.tile([128, cfg.HO], BF16, tag="h2b")
                            nc.vector.scalar_tensor_tensor(
                                out=h2b[:], in0=ex[:], scalar=-1.0, in1=rl[:],
                                op0=ALU.add, op1=ALU.add)
                            tp2 = pstr.tile([128, 128], BF16, tag="tp")
                            nc.tensor.transpose(tp2[:], h2b[:], ident[:])
                            h2T = wk.tile([128, 128], BF16, tag="h2T")
                            nc.scalar.activation(h2T[:], tp2[:], AF.Copy)
                            ps3 = psx.tile([128, cfg.N_CLS + 2], F32, tag="ps3")
                            nc.tensor.matmul(ps3[:], lhsT=h2T[:], rhs=w2_t[:], start=True, stop=True)
                            l2r = wk.tile([128, cfg.L2W], BF16, tag="l2r")
                            nc.scalar.activation(l2r[:, 0:cfg.N_CLS], ps3[:, 0:cfg.N_CLS], AF.Copy)
                            nc.vector.memset(l2r[:, cfg.N_CLS:cfg.N_CLS + 1], 1.0)
                            nc.scalar.activation(l2r[:, cfg.N_CLS + 1:cfg.N_CLS + 2],
                                                 ps3[:, cfg.N_CLS:cfg.N_CLS + 1], AF.Copy)
                            nc.scalar.activation(ad2all[:, b:b + 1],
                                                 ps3[:, cfg.N_CLS + 1:cfg.N_CLS + 2], AF.Copy)
                            nc.sync.dma_start(l2own[b * 128:(b + 1) * 128, 0:cfg.L2W], l2r[:])
                        else:
                            zt2 = scp.tile([128, 1], F32, tag="zt2")
                            nc.vector.tensor_scalar_max(zt2[:], pblk[:, cfg.N_CLS:cfg.N_CLS + 1], 1e-30)
                            rz2 = scp.tile([128, 1], F32, tag="rz2")
                            nc.vector.reciprocal(rz2[:], zt2[:])
                            of = wk.tile([128, cfg.N_CLS], F32, tag="of")
                            nc.scalar.activation(of[:], pblk[:, 0:cfg.N_CLS], AF.Copy,
                                                 scale=rz2[:, 0:1])
                            nc.sync.dma_start(out[b * 128:(b + 1) * 128, :], of[:])

            edge_layer(1)
            # ---------------- P2: AllGather L2 table ----------------------
            nc.gpsimd.collective_compute(
                "AllGather", ALU.bypass,
                replica_groups=[list(range(cfg.n_cores))],
                ins=[l2own[:]], outs=[l2tab[:]])
            hi_elems = (cfg.NG - cfg.SPLIT) * cfg.L2ROW
            CH = 16384
            assert hi_elems % CH == 0, hi_elems
            nc.sync.dma_start(
                l2hi[:, :].flatten().rearrange("(a b) -> a b", b=CH),
                l2tab[:, :].flatten()[cfg.SPLIT * cfg.L2ROW:].rearrange("(a b) -> a b", b=CH))
            edge_layer(2)

    nc.compile()
    if legalize:
        legalize_waits(nc)
    return nc


# ------------------------------------------------------------ numpy ref ---
def numpy_ref(cfg, inputs):
    x = np.asarray(inputs["x"], np.float32)
    W1 = np.asarray(inputs["W1"], np.float32)
    as1 = np.asarray(inputs["att_src1"], np.float32)
    ad1 = np.asarray(inputs["att_dst1"], np.float32)
    W2 = np.asarray(inputs["W2"], np.float32)
    as2 = np.asarray(inputs["att_src2"], np.float32)
    ad2 = np.asarray(inputs["att_dst2"], np.float32)
    ei = np.asarray(inputs["edge_index"])
    N = cfg.N
    loops = np.arange(N, dtype=np.int64)
    src = np.concatenate([ei[0].astype(np.int64), loops])
    dst = np.concatenate([ei[1].astype(np.int64), loops])

    def lrelu(v):
        return np.where(v > 0, v, cfg.neg_slope * v)

    def gat(x, W, att_s, att_d, heads, out_ch):
        h = (x @ W).reshape(N, heads, out_ch)
        a_s = (h * att_s).sum(-1)
        a_d = (h * att_d).sum(-1)
        e = lrelu(a_s[src] + a_d[dst])
        ex = np.exp(e)
        z = np.zeros((N, heads), np.float32)
        np.add.at(z, dst, ex)
        alpha = ex / z[dst]
        msg = h[src] * alpha[:, :, None]
        o = np.zeros((N, heads, out_ch), np.float32)
        np.add.at(o, dst, msg)
        return o.reshape(N, heads * out_ch)

    h = gat(x, W1, as1, ad1, cfg.HEADS, cfg.HID)
    h = np.where(h > 0, h, np.exp(np.minimum(h, 0)) - 1)
    o = gat(h, W2, as2, ad2, 1, cfg.N_CLS)
    return o


def assemble_output(cfg, shared, results):
    full = np.zeros((cfg.N, cfg.N_CLS), np.float32)
    for k in range(cfg.n_cores):
        perm = shared["metas"][k]["perm"]
        o = results[k]["out"]  # [NPCP, 40]
        for b in range(cfg.n_blocks):
            for j in range(128):
                v = perm[b, j]
                if v >= 0:
                    full[k * cfg.NPC + v] = o[b * 128 + j]
    return full


# ================================================================ entry ===
_CACHE = {}

def kernel(**inputs):
    """Full-input GAT kernel: shards across 8 TRN2 NeuronCores internally."""
    cfg = Cfg()  # problem-size defaults
    install_ntff_hook()
    from concourse.bass_utils import run_bass_kernel_spmd
    per_core, shared = prep_host(cfg, inputs)
    key = (cfg.C_lo, cfg.C_hi, cfg.C2_lo, cfg.C2_hi)
    if key not in _CACHE:
        _CACHE[key] = build_kernel(cfg, shared)
    nc = _CACHE[key]
    want_trace = os.environ.get("GAT_TRACE", "1") == "1"
    try:
        res = run_bass_kernel_spmd(nc, per_core, core_ids=list(range(cfg.n_cores)),
                                   trace=want_trace)
    except Exception:
        if not want_trace:
            raise
        res = run_bass_kernel_spmd(nc, per_core, core_ids=list(range(cfg.n_cores)),
                                   trace=False)
    global LAST_EXEC_NS
    LAST_EXEC_NS = res.exec_time_ns
    return assemble_output(cfg, shared, res.results)


LAST_EXEC_NS = None



# revision 5
# speedup vs baseline: 1.1618x; 1.1618x over previous
"""GAT (2 layers: 4 heads x 32, then 1 head x 40) on 8 TRN2 NeuronCores.

Sharding: dst-node sharding across 8 cores. Per core, its NPC nodes are
host-packed into `n_blocks` blocks of <=128 nodes, balanced so that every
block's incoming-edge count (split into lo/hi src streams for int16
dma_gather addressing) fits a FIXED number of 128-edge chunks (C_lo / C_hi)
-- this makes the SPMD program structurally identical across cores while all
per-core irregularity lives in the input data (gather indices, dst-slot ids,
permuted x slices).

Per-edge pipeline (one 128-edge chunk):
  dma_gather slab rows [h1 interleaved with 1.0 cols | a_s] by src;
  D[e,d] = onehot(dst_slot) via DVE tensor_scalar is_equal vs iota;
  D_T via PE transpose; a_d per edge = D_T.T @ a_d_block (PE);
  w = exp(leaky_relu(a_s + a_d)) (DVE add, ACT lrelu, ACT exp+broadcast);
  Msg = G * w_exp (DVE); psum_blk += D.T @ Msg (PE) -- the interleaved 1.0
  columns turn into the softmax normalizers z_h.
Block epilogue: h1 = psum/z, h2 = elu(h1), transpose, @ [W2|ws2|wd2] -> L2
table row [h2p(40) | 1.0 | a_s2] + resident a_d2. AllGather L2 table, run the
same edge pipeline for layer 2, out = psum2/z2.
"""
import sys, os, types, time
sys.path.insert(0, '/opt/trn_rl_repo')
import numpy as np
import ml_dtypes

import concourse.bass as bass
import concourse.bacc as bacc
import concourse.mybir as mybir
import concourse.tile as tile
from concourse.vector_clock import ScopedClock
from concourse.masks import make_identity

BF16 = mybir.dt.bfloat16
F32 = mybir.dt.float32
AF = mybir.ActivationFunctionType
ALU = mybir.AluOpType
NPBF = ml_dtypes.bfloat16

# ---------------------------------------------------------------- compat ---
_MAXW = 1

def _patched_drain_and_barrier(self, tick_clock, wait_clock):
    nc = self.nc
    drain_inst = nc.sync.drain()
    wait_clock.add_sem_waits(drain_inst.ins, ScopedClock({None: tick_clock.global_clock}))
    si = drain_inst.ins.sync_info
    waits = list(si.on_wait or [])
    if len(waits) > _MAXW:
        si.on_wait.clear()
        si.on_wait.extend(waits[:_MAXW])
        rest = waits[_MAXW:]
        for i in range(0, len(rest), _MAXW):
            extra = nc.sync.drain()
            esi = extra.ins.sync_info
            if esi is None:
                extra.ins.sync_info = mybir.SyncInfo(on_wait=[], on_update=[])
                esi = extra.ins.sync_info
            esi.on_wait.extend(rest[i:i + _MAXW])
    nc.all_engine_barrier()
    assert self.sems is not None
    popped = nc._tile_sem_poison_stack.pop()
    assert popped is self._sem_poison
    nc.clear_and_free_semaphores(list(self.sems.allocated().values()))
    nc.all_engine_barrier()

tile.TileContext._drain_and_barrier = _patched_drain_and_barrier


def legalize_waits(nc, maxw=_MAXW):
    n_split = 0
    for fn in nc.m.functions:
        for bb in fn.blocks:
            out = []
            for ins in bb.instructions:
                si = ins.sync_info
                waits = list(si.on_wait) if (si is not None and si.on_wait) else []
                if len(waits) > maxw:
                    extras, keep = waits[:-maxw], waits[-maxw:]
                    for j, w in enumerate(extras):
                        nop = mybir.InstEventSemaphore(name=f"{ins.name}_xw{j}", ins=[], outs=[])
                        nop.engine = ins.engine
                        nop.sync_info = mybir.SyncInfo(on_wait=[w], on_update=[])
                        out.append(nop)
                        n_split += 1
                    si.on_wait.clear()
                    si.on_wait.extend(keep)
                out.append(ins)
            bb.instructions[:] = out
    return n_split


def install_ntff_hook():
    if "antenv.axon_hooks" in sys.modules:
        return
    m = types.ModuleType("antenv.axon_hooks")
    _hook = [None]
    m.set_axon_ntff_profile_hook = lambda h: _hook.__setitem__(0, h)
    m.get_axon_ntff_profile_hook = lambda: _hook[0]
    sys.modules["antenv.axon_hooks"] = m
    import antenv
    antenv.axon_hooks = m
    try:
        from trn_agent_boot.trn_boot import _ntff_profile_via_ctypes
        m.set_axon_ntff_profile_hook(_ntff_profile_via_ctypes('/opt/axon/libaxon_pjrt.so'))
    except Exception:
        pass


# ---------------------------------------------------------------- config ---
class Cfg:
    def __init__(self, N=50000, F_IN=128, HID=32, HEADS=4, N_CLS=40, E=800000,
                 n_cores=8, slab_chunks=24, neg_slope=0.2, split=32768):
        self.N, self.F_IN, self.HID, self.HEADS, self.N_CLS, self.E = N, F_IN, HID, HEADS, N_CLS, E
        self.n_cores = n_cores
        self.SC = slab_chunks                 # chunks per gather slab
        self.neg_slope = neg_slope
        assert N % n_cores == 0
        self.NPC = N // n_cores
        self.n_blocks = (self.NPC + 127) // 128
        self.NPCP = self.n_blocks * 128       # padded nodes per core
        self.HO = HEADS * HID                 # 128
        self.L1C = self.HO + HEADS            # 132 msg cols (features + z cols)
        self.L1W = self.L1C + HEADS           # 136 written cols (plus a_s)
        self.L1ROW = 256                      # bf16 elems per table row (512B)
        self.L2C = N_CLS + 1                  # 41 msg cols (h2p + z col)
        self.L2W = N_CLS + 2                  # 42 written cols (plus a_s2)
        self.L2ROW = 128                      # bf16 per row (256B)
        self.SPLIT = split
        assert split % 128 == 0
        self.NP = ((N + 127) // 128) * 128    # padded node count (dense tiles)
        self.NG = self.n_cores * self.NPCP    # global permuted L2 table rows
        # filled by prep: C_lo, C_hi


# ------------------------------------------------------------ host prep ---
def _pack_blocks(cfg, deg_lo, deg_hi):
    """Greedy balance nodes (local ids 0..NPC-1) into n_blocks blocks of
    <=128 nodes minimizing the max lo/hi edge load. Returns perm [n_blocks,128]
    (local node id or -1)."""
    nb = cfg.n_blocks
    order = np.argsort(-(deg_lo + deg_hi), kind='stable')
    cnt = np.zeros(nb, np.int64)
    lo = np.zeros(nb, np.int64)
    hi = np.zeros(nb, np.int64)
    perm = -np.ones((nb, 128), np.int64)
    for v in order:
        w = deg_lo[v] + deg_hi[v]
        # pick feasible block with min total load
        feas = np.where(cnt < 128)[0]
        b = feas[np.argmin(lo[feas] + hi[feas])]
        perm[b, cnt[b]] = v
        cnt[b] += 1
        lo[b] += deg_lo[v]
        hi[b] += deg_hi[v]
    return perm, lo, hi


def prep_core(cfg, src, dst, core):
    """Per-core packing. Returns dict with perm, per-(block,stream) edge
    arrays (src-sorted), and counts."""
    base = core * cfg.NPC
    m = (dst >= base) & (dst < base + cfg.NPC)
    s, d = src[m], dst[m] - base
    deg_lo = np.bincount(d[s < cfg.SPLIT], minlength=cfg.NPC)
    deg_hi = np.bincount(d[s >= cfg.SPLIT], minlength=cfg.NPC)
    perm, lo_sum, hi_sum = _pack_blocks(cfg, deg_lo, deg_hi)
    # node -> (block, slot)
    blk_of = -np.ones(cfg.NPC, np.int64)
    slot_of = -np.ones(cfg.NPC, np.int64)
    for b in range(cfg.n_blocks):
        for j in range(128):
            v = perm[b, j]
            if v >= 0:
                blk_of[v] = b
                slot_of[v] = j
    eb = blk_of[d]
    es = slot_of[d]
    per = {}
    for b in range(cfg.n_blocks):
        mb = eb == b
        sb, slb = s[mb], es[mb]
        for st in (0, 1):
            ms = (sb < cfg.SPLIT) if st == 0 else (sb >= cfg.SPLIT)
            ss, sl = sb[ms], slb[ms]
            o = np.argsort(ss, kind='stable')
            per[(b, st)] = (ss[o], sl[o])
    return {"perm": perm, "per": per,
            "c_lo": int(np.max(np.ceil(lo_sum / 128))) if len(lo_sum) else 0,
            "c_hi": int(np.max(np.ceil(hi_sum / 128))) if len(hi_sum) else 0}


def build_streams(cfg, pc, l2_bases):
    """Given per-core packing pc and fixed C_lo/C_hi, build the gather idx
    arrays and dst-slot arrays for L1 (raw node ids) and L2 (permuted global
    ids via l2_bases[node] = permuted row)."""
    C = {0: cfg.C_lo, 1: cfg.C_hi}
    out = {}
    for layer, xl in (("l1", None), ("l2", l2_bases)):
        for st in (0, 1):
            nchunks = cfg.n_blocks * C[st]
            idx = np.zeros((nchunks, 128), np.int64)
            dl = np.full((nchunks, 128), 255, np.int64)
            ci = 0
            for b in range(cfg.n_blocks):
                ss, sl = pc["per"][(b, st)]
                if layer == "l2":
                    g = xl[ss]                       # permuted global rows
                    g = np.sort(g)                    # locality; slot must follow!
                    # NOTE: need slots aligned with sorted order:
                    o = np.argsort(xl[ss], kind='stable')
                    g, sl2 = xl[ss][o], sl[o]
                    base_off = cfg.SPLIT if st == 1 else 0
                    # re-split: permuted ids may cross SPLIT differently!
                    # handled by caller building separate l2 streams.
                    v = g - base_off
                    svals, slvals = v, sl2
                else:
                    svals = ss - (cfg.SPLIT if st == 1 else 0)
                    slvals = sl
                n = len(svals)
                cap = C[st] * 128
                assert n <= cap, (n, cap)
                a = np.zeros(cap, np.int64)
                a[:n] = svals
                dd = np.full(cap, 255, np.int64)
                dd[:n] = slvals
                for c in range(C[st]):
                    idx[ci] = a[c * 128:(c + 1) * 128]
                    dl[ci] = dd[c * 128:(c + 1) * 128]
                    ci += 1
            out[(layer, st)] = (idx, dl)
    return out


def prep_host(cfg, inputs):
    x = np.asarray(inputs["x"], np.float32)
    W1 = np.asarray(inputs["W1"], np.float32)
    as1 = np.asarray(inputs["att_src1"], np.float32)
    ad1 = np.asarray(inputs["att_dst1"], np.float32)
    W2 = np.asarray(inputs["W2"], np.float32)
    as2 = np.asarray(inputs["att_src2"], np.float32)
    ad2 = np.asarray(inputs["att_dst2"], np.float32)
    ei = np.asarray(inputs["edge_index"])
    loops = np.arange(cfg.N, dtype=np.int64)
    src = np.concatenate([ei[0].astype(np.int64), loops])
    dst = np.concatenate([ei[1].astype(np.int64), loops])

    H, HID = cfg.HEADS, cfg.HID
    # W1il: interleaved feature cols with zero cols at the z positions
    W1r = W1.reshape(cfg.F_IN, H, HID)
    W1il = np.zeros((cfg.F_IN, cfg.L1C), np.float32)
    for h in range(H):
        W1il[:, 33 * h:33 * h + HID] = W1r[:, h]
    Ws1 = (W1r * as1[None]).sum(-1)
    Wd1 = (W1r * ad1[None]).sum(-1)
    W1ext = np.concatenate([W1il, Ws1, Wd1], 1).astype(NPBF)   # [128, 140]
    ws2 = (W2 * as2).sum(1, keepdims=True)
    wd2 = (W2 * ad2).sum(1, keepdims=True)
    W2ext = np.concatenate([W2, ws2, wd2], 1).astype(NPBF)     # [128, 42]

    xT = np.zeros((cfg.F_IN, cfg.NP), NPBF)
    xT[:, :cfg.N] = np.ascontiguousarray(x.T).astype(NPBF)
    iota = np.broadcast_to(np.arange(128, dtype=np.float32), (128, 128)).astype(NPBF)

    # pass 1: per-core packing, find global C_lo/C_hi
    pcs = [prep_core(cfg, src, dst, k) for k in range(cfg.n_cores)]
    cfg.C_lo = max(p["c_lo"] for p in pcs)
    cfg.C_hi = max(p["c_hi"] for p in pcs)

    # L2 permuted global row of node v
    l2_bases = np.zeros(cfg.N, np.int64)
    for k in range(cfg.n_cores):
        perm = pcs[k]["perm"]
        for b in range(cfg.n_blocks):
            for j in range(128):
                v = perm[b, j]
                if v >= 0:
                    l2_bases[k * cfg.NPC + v] = k * cfg.NPCP + b * 128 + j

    # L2 streams must be re-split by PERMUTED id (lo: <SPLIT, hi: >=SPLIT).
    # Simplest correct approach: recompute per-core per-block streams in the
    # permuted-id space with its own balance constraint... but the block
    # structure (C_lo/C_hi chunks) is shared with L1. Instead: keep the SAME
    # chunk structure (same edge->chunk assignment as L1) and allow l2 idx
    # to go out of int16 range? No. Resolution: build L2 edge streams
    # independently (re-sorted by permuted src id and re-split lo/hi), with
    # their own fixed chunk counts C2_lo/C2_hi.
    per2 = []
    c2lo = c2hi = 0
    for k in range(cfg.n_cores):
        base = k * cfg.NPC
        m = (dst >= base) & (dst < base + cfg.NPC)
        s0, d0 = src[m], dst[m] - base
        perm = pcs[k]["perm"]
        blk_of = -np.ones(cfg.NPC, np.int64)
        slot_of = -np.ones(cfg.NPC, np.int64)
        for b in range(cfg.n_blocks):
            for j in range(128):
                v = perm[b, j]
                if v >= 0:
                    blk_of[v] = b
                    slot_of[v] = j
        g = l2_bases[s0]                       # permuted global src row
        eb, es = blk_of[d0], slot_of[d0]
        per = {}
        for b in range(cfg.n_blocks):
            mb = eb == b
            gb, sb = g[mb], es[mb]
            for st in (0, 1):
                ms = (gb < cfg.SPLIT) if st == 0 else (gb >= cfg.SPLIT)
                gg, sl = gb[ms], sb[ms]
                o = np.argsort(gg, kind='stable')
                per[(b, st)] = (gg[o], sl[o])
                n = len(gg)
                if st == 0:
                    c2lo = max(c2lo, int(np.ceil(n / 128)))
                else:
                    c2hi = max(c2hi, int(np.ceil(n / 128)))
        per2.append(per)
    cfg.C2_lo, cfg.C2_hi = c2lo, c2hi

    def pack_stream(per, b_range, st, C, base_off):
        nchunks = cfg.n_blocks * C
        idx = np.zeros((nchunks, 128), np.int64)
        dl = np.full((nchunks, 128), 255, np.int64)
        ci = 0
        for b in b_range:
            ss, sl = per[(b, st)]
            n = len(ss)
            cap = C * 128
            assert n <= cap
            a = np.zeros(cap, np.int64)
            a[:n] = ss - base_off
            dd = np.full(cap, 255, np.int64)
            dd[:n] = sl
            for c in range(C):
                idx[ci] = a[c * 128:(c + 1) * 128]
                dl[ci] = dd[c * 128:(c + 1) * 128]
                ci += 1
        return idx, dl

    def to_gather_layout(cfg, idx_chunks, dl_chunks, SC):
        """idx_chunks [nch, 128] -> slabs of SC chunks:
        idx16 [nslab, 128, SC*8] (i at partition i%16, free i//16, 8x stripes)
        dl bf16 [nslab, 128, SC]."""
        nch = idx_chunks.shape[0]
        nslab = (nch + SC - 1) // SC
        idx16 = np.zeros((nslab, 128, SC * 8), np.int16)
        dlb = np.full((nslab, 128, SC), 255.0, np.float32)
        sizes = []
        for si in range(nslab):
            c0, c1 = si * SC, min((si + 1) * SC, nch)
            n = (c1 - c0) * 128
            flat = idx_chunks[c0:c1].reshape(-1)   # chunk-major, i = c*128+p
            w = flat.astype(np.int16).reshape(-1, 16).T  # [16, n/16]
            idx16[si, :, :w.shape[1]] = np.tile(w, (8, 1))
            dlb[si, :, :c1 - c0] = dl_chunks[c0:c1].T.astype(np.float32)
            sizes.append(n)
        return idx16, dlb, sizes

    per_core, metas = [], []
    for k in range(cfg.n_cores):
        pc, p2 = pcs[k], per2[k]
        im = {"xT": np.asarray(xT), "W1ext": np.asarray(W1ext),
              "W2ext": np.asarray(W2ext), "iota": np.asarray(iota)}
        # permuted own x slice (padded slots -> zero cols)
        xo = np.zeros((cfg.F_IN, cfg.NPCP), NPBF)
        perm = pc["perm"]
        for b in range(cfg.n_blocks):
            for j in range(128):
                v = perm[b, j]
                if v >= 0:
                    xo[:, b * 128 + j] = xT[:, k * cfg.NPC + v]
        im["xT_own"] = xo
        meta = {"perm": perm}
        for tag, per, Clo, Chi, split in (
                ("a", pc["per"], cfg.C_lo, cfg.C_hi, cfg.SPLIT),
                ("b", p2, cfg.C2_lo, cfg.C2_hi, cfg.SPLIT)):
            for st, C in ((0, Clo), (1, Chi)):
                idxc, dlc = pack_stream(per, range(cfg.n_blocks), st, C, split if st else 0)
                i16, dlb, sizes = to_gather_layout(cfg, idxc, dlc, cfg.SC)
                im[f"idx_{tag}{st}"] = i16
                im[f"dl_{tag}{st}"] = np.asarray(dlb)
                meta[f"sizes_{tag}{st}"] = sizes
        per_core.append(im)
        metas.append(meta)
    shared = {"metas": metas, "l2_bases": l2_bases, "pcs": pcs}
    return per_core, shared


# ------------------------------------------------------------ device build ---
def build_kernel(cfg, shared, legalize=True):
    SC = cfg.SC
    sizes_a0 = shared["metas"][0]["sizes_a0"]
    sizes_a1 = shared["metas"][0]["sizes_a1"]
    sizes_b0 = shared["metas"][0]["sizes_b0"]
    sizes_b1 = shared["metas"][0]["sizes_b1"]
    ns_a0, ns_a1 = len(sizes_a0), len(sizes_a1)
    ns_b0, ns_b1 = len(sizes_b0), len(sizes_b1)
    NT = cfg.NP // 128
    H = cfg.HEADS

    nc = bacc.Bacc("TRN2", target_bir_lowering=False, num_devices=cfg.n_cores)

    xT = nc.dram_tensor("xT", [cfg.F_IN, cfg.NP], BF16, kind="ExternalInput")
    xT_own = nc.dram_tensor("xT_own", [cfg.F_IN, cfg.NPCP], BF16, kind="ExternalInput")
    W1ext = nc.dram_tensor("W1ext", [cfg.F_IN, cfg.L1C + 2 * H], BF16, kind="ExternalInput")
    W2ext = nc.dram_tensor("W2ext", [cfg.HO, cfg.N_CLS + 2], BF16, kind="ExternalInput")
    iota_in = nc.dram_tensor("iota", [128, 128], BF16, kind="ExternalInput")
    gin = {}
    for tag, ns in (("a0", ns_a0), ("a1", ns_a1), ("b0", ns_b0), ("b1", ns_b1)):
        gin[f"idx_{tag}"] = nc.dram_tensor(f"idx_{tag}", [max(ns, 1), 128, SC * 8], mybir.dt.int16, kind="ExternalInput")
        gin[f"dl_{tag}"] = nc.dram_tensor(f"dl_{tag}", [max(ns, 1), 128, SC], F32, kind="ExternalInput")
    out = nc.dram_tensor("out", [cfg.NPCP, cfg.N_CLS], F32, kind="ExternalOutput")

    l1lo = nc.dram_tensor("l1lo", [cfg.SPLIT, cfg.L1ROW], BF16)
    l1hi = nc.dram_tensor("l1hi", [cfg.NP - cfg.SPLIT, cfg.L1ROW], BF16)
    l2own = nc.dram_tensor("l2own", [cfg.NPCP, cfg.L2ROW], BF16)
    l2tab = nc.dram_tensor("l2tab", [cfg.NG, cfg.L2ROW], BF16, addr_space="Shared")
    l2hi = nc.dram_tensor("l2hi", [cfg.NG - cfg.SPLIT, cfg.L2ROW], BF16)

    with tile.TileContext(nc) as tc:
        regcache = {}
        def reg(v):
            if v not in regcache:
                regcache[v] = nc.gpsimd.to_reg(v)
            return regcache[v]

        with tc.tile_pool(name="consts", bufs=1) as consts:
            iota_t = consts.tile([128, 128], BF16)
            nc.sync.dma_start(iota_t[:], iota_in[:])
            ident = consts.tile([128, 128], BF16)
            make_identity(nc, ident[:])
            w1_t = consts.tile([cfg.F_IN, cfg.L1C + 2 * H], BF16)
            nc.sync.dma_start(w1_t[:], W1ext[:])
            w2_t = consts.tile([cfg.HO, cfg.N_CLS + 2], BF16)
            nc.sync.dma_start(w2_t[:], W2ext[:])
            adall = consts.tile([128, cfg.n_blocks * H], BF16)
            ad2all = consts.tile([128, cfg.n_blocks], BF16)

            # ---------------- P0: dense L1 table over all nodes ----------
            # 3 node-tiles per group: 3 matmuls into one PSUM bank, one ACT
            # copy, one strided memset, one (or two, at the lo/hi boundary)
            # table DMA -- cuts ACT + DMA instruction counts ~3x.
            GT = 3
            tl = cfg.SPLIT // 128
            with (
                tc.tile_pool(name="p0", bufs=4) as p0pool,
                tc.tile_pool(name="p0ps", bufs=2, space="PSUM") as p0ps,
            ):
                for gi in range((NT + GT - 1) // GT):
                    t0 = gi * GT
                    nh = min(GT, NT - t0)
                    xt = p0pool.tile([cfg.F_IN, GT * 128], BF16, tag="xt")
                    eng_in = nc.scalar if (gi % 2) else nc.sync
                    eng_in.dma_start(xt[:, 0:nh * 128], xT[:, t0 * 128:(t0 + nh) * 128])
                    ps = p0ps.tile([128, GT, cfg.L1W], F32, tag="p0ps")
                    for j in range(nh):
                        nc.tensor.matmul(ps[:, j, :], lhsT=xt[:, j * 128:(j + 1) * 128],
                                         rhs=w1_t[:, 0:cfg.L1W], start=True, stop=True)
                    ev = p0pool.tile([128, GT, cfg.L1W], BF16, tag="ev")
                    nc.scalar.activation(ev[:, 0:nh, :], ps[:, 0:nh, :], AF.Copy)
                    nc.vector.memset(ev[:, 0:nh, 32:cfg.L1C:33], 1.0)
                    weng = nc.sync if (gi % 2) else nc.scalar
                    jlo = max(0, min(nh, tl - t0))
                    if jlo > 0:
                        weng.dma_start(
                            l1lo[t0 * 128:(t0 + jlo) * 128, 0:cfg.L1W]
                            .rearrange("(j p) c -> p j c", p=128),
                            ev[:, 0:jlo, :])
                    if jlo < nh:
                        th0 = t0 + jlo - tl
                        weng.dma_start(
                            l1hi[th0 * 128:(th0 + nh - jlo) * 128, 0:cfg.L1W]
                            .rearrange("(j p) c -> p j c", p=128),
                            ev[:, jlo:nh, :])
                # own-range a_d (permuted block layout), 4 blocks per DMA/copy
                for b0 in range(0, cfg.n_blocks, 4):
                    nb4 = min(4, cfg.n_blocks - b0)
                    xo = p0pool.tile([cfg.F_IN, 512], BF16, tag="xt")
                    nc.sync.dma_start(xo[:, 0:nb4 * 128],
                                      xT_own[:, b0 * 128:(b0 + nb4) * 128])
                    ps = p0ps.tile([128, 4, H], F32, tag="p0psb")
                    for j in range(nb4):
                        nc.tensor.matmul(ps[:, j, :], lhsT=xo[:, j * 128:(j + 1) * 128],
                                         rhs=w1_t[:, cfg.L1C + H:cfg.L1C + 2 * H],
                                         start=True, stop=True)
                    nc.scalar.activation(adall[:, H * b0:H * (b0 + nb4)], ps[:, 0:nb4, :], AF.Copy)

            # ---------------- edge-layer template -------------------------
            def edge_layer(layer):
                if layer == 1:
                    tab_lo, tab_hi, tag_lo, tag_hi = l1lo, l1hi, "a0", "a1"
                    C_lo, C_hi = cfg.C_lo, cfg.C_hi
                    sizes = {0: sizes_a0, 1: sizes_a1}
                    ROW, MC = cfg.L1ROW, cfg.L1C
                else:
                    tab_lo, tab_hi, tag_lo, tag_hi = l2tab, l2hi, "b0", "b1"
                    C_lo, C_hi = cfg.C2_lo, cfg.C2_hi
                    sizes = {0: sizes_b0, 1: sizes_b1}
                    ROW, MC = cfg.L2ROW, cfg.L2C
                NH = H if layer == 1 else 1

                with (
                    tc.tile_pool(name=f"gsl{layer}", bufs=3) as gpool_lo,
                    tc.tile_pool(name=f"gsh{layer}", bufs=3) as gpool_hi,
                    tc.tile_pool(name=f"dl{layer}", bufs=3) as dlpool,
                    tc.tile_pool(name=f"ix{layer}", bufs=3) as ixpool,
                    tc.tile_pool(name=f"wk{layer}", bufs=4) as wk,
                    tc.tile_pool(name=f"dp{layer}", bufs=3) as dp,
                    tc.tile_pool(name=f"dtp{layer}", bufs=4) as dtp,
                    tc.tile_pool(name=f"sc{layer}", bufs=4) as scp,
                    tc.tile_pool(name=f"ps{layer}", bufs=2, space="PSUM") as psacc,
                    tc.tile_pool(name=f"pst{layer}", bufs=2, space="PSUM") as pstr,
                    tc.tile_pool(name=f"psx{layer}", bufs=2, space="PSUM") as psx,
                ):
                    cur = {0: -1, 1: -1}
                    slabs = {0: None, 1: None}
                    dls = {0: None, 1: None}

                    def ensure_slab(st, si):
                        if cur[st] == si:
                            return
                        cur[st] = si
                        pool = gpool_lo if st == 0 else gpool_hi
                        tname = tag_lo if st == 0 else tag_hi
                        it = ixpool.tile([128, SC * 8], mybir.dt.int16, tag=f"it{st}")
                        nc.sync.dma_start(it[:], gin[f"idx_{tname}"][si])
                        dlt = dlpool.tile([128, SC], F32, tag=f"dl{st}")
                        nc.sync.dma_start(dlt[:], gin[f"dl_{tname}"][si])
                        g = pool.tile([128, SC, ROW], BF16, tag=f"g{st}")
                        n = sizes[st][si]
                        tabx = tab_lo if st == 0 else tab_hi
                        nc.gpsimd.dma_gather(
                            out_ap=g[:, 0:max(n // 128, 1), :],
                            in_ap=tabx[:, :],
                            idxs_ap=it[:, 0:max(n // 16, 1)],
                            num_idxs=n, num_idxs_reg=reg(n),
                            elem_size=ROW, single_packet=False)
                        slabs[st] = g
                        dls[st] = dlt

                    for b in range(cfg.n_blocks):
                        pblk = psacc.tile([128, MC], F32, tag="acc")
                        first = True
                        # chunk list: (stream, global chunk id)
                        chs = [(0, b * C_lo + c) for c in range(C_lo)] + \
                              [(1, b * C_hi + c) for c in range(C_hi)]
                        # group for score batching: consecutive chunks in same slab
                        groups = []
                        gcur = None
                        for st, gc in chs:
                            si, col = gc // SC, gc % SC
                            if gcur and gcur[0] == st and gcur[1] == si and len(gcur[2]) < 12:
                                gcur[2].append(col)
                            else:
                                gcur = [st, si, [col]]
                                groups.append(gcur)
                        nch = len(chs)
                        ci_total = 0
                        for st, si, cols in groups:
                            ensure_slab(st, si)
                            g, dlt = slabs[st], dls[st]
                            ng = len(cols)
                            c0 = cols[0]
                            # per-chunk one-hot builds into one group tile
                            Dg = dp.tile([128, 12, 128], BF16, tag="D")
                            for j, col in enumerate(cols):
                                nc.vector.tensor_scalar(
                                    Dg[:, j, :], iota_t[:], dlt[:, col:col + 1], None,
                                    ALU.is_equal)
                            # transposes in sub-batches of 4, one PSUM->SBUF copy each
                            adx = psx.tile([128, 12 * NH], F32, tag="adx")
                            for q0 in range(0, ng, 4):
                                qn = min(4, ng - q0)
                                tp4 = pstr.tile([128, 4, 128], BF16, tag="tp")
                                for q in range(qn):
                                    nc.tensor.transpose(tp4[:, q, :], Dg[:, q0 + q, :], ident[:])
                                DT4 = dtp.tile([128, 4, 128], BF16, tag="DT")
                                nc.scalar.activation(DT4[:, 0:qn, :], tp4[:, 0:qn, :], AF.Copy)
                                for q in range(qn):
                                    nc.tensor.matmul(
                                        adx[:, (q0 + q) * NH:(q0 + q + 1) * NH],
                                        lhsT=DT4[:, q, :],
                                        rhs=(adall[:, H * b:H * b + H] if layer == 1
                                             else ad2all[:, b:b + 1]),
                                        start=True, stop=True)
                            # batched scores for the group
                            sc_t = scp.tile([128, 12 * NH], F32, tag="sc")
                            if layer == 1:
                                asrc = g[:, c0:c0 + ng, cfg.L1C:cfg.L1C + H]
                            else:
                                asrc = g[:, c0:c0 + ng, cfg.N_CLS + 1:cfg.N_CLS + 2]
                            nc.vector.tensor_tensor(
                                out=sc_t[:, 0:ng * NH], in0=adx[:, 0:ng * NH],
                                in1=asrc, op=ALU.add)
                            nc.vector.scalar_tensor_tensor(
                                out=sc_t[:, 0:ng * NH], in0=sc_t[:, 0:ng * NH],
                                scalar=cfg.neg_slope, in1=sc_t[:, 0:ng * NH],
                                op0=ALU.mult, op1=ALU.max)
                            msg = wk.tile([128, 12, MC], BF16, tag="msg")
                            if layer == 1:
                                # one broadcast-exp for the whole group, then one
                                # batched per-edge multiply over [128, ng, 132]
                                wex = wk.tile([128, 12, cfg.L1C], BF16, tag="wex")
                                nc.scalar.activation(
                                    wex[:, 0:ng, :].rearrange("p g (h q) -> p (g h) q", q=33),
                                    sc_t[:, 0:ng * H].to_broadcast([128, ng * H, 33]),
                                    AF.Exp)
                                nc.vector.tensor_tensor(
                                    out=msg[:, 0:ng, :], in0=g[:, c0:c0 + ng, 0:MC],
                                    in1=wex[:, 0:ng, :], op=ALU.mult)
                            else:
                                # per-partition scale path on ACT (frees DVE)
                                w2s = scp.tile([128, 12], F32, tag="w2s")
                                nc.scalar.activation(w2s[:, 0:ng], sc_t[:, 0:ng], AF.Exp)
                                for j in range(ng):
                                    nc.scalar.activation(msg[:, j, :], g[:, c0 + j, 0:MC],
                                                         AF.Copy, scale=w2s[:, j:j + 1])
                            for j, col in enumerate(cols):
                                ci_total += 1
                                nc.tensor.matmul(pblk[:], lhsT=Dg[:, j, :], rhs=msg[:, j, :],
                                                 start=first, stop=(ci_total == nch))
                                first = False
                        # -------- block epilogue --------
                        if layer == 1:
                            zt = scp.tile([128, H], F32, tag="zt")
                            nc.vector.tensor_scalar_max(zt[:], pblk[:, 32:cfg.L1C:33], 1e-30)
                            rz = scp.tile([128, H], F32, tag="rz")
                            nc.vector.reciprocal(rz[:], zt[:])
                            h1f = wk.tile([128, cfg.HO], F32, tag="h1f")
                            for h in range(H):
                                nc.scalar.activation(
                                    h1f[:, 32 * h:32 * h + 32],
                                    pblk[:, 33 * h:33 * h + 32],
                                    AF.Copy, scale=rz[:, h:h + 1])
                            mn = wk.tile([128, cfg.HO], F32, tag="mn")
                            nc.vector.tensor_scalar_min(mn[:], h1f[:], 0.0)
                            ex = wk.tile([128, cfg.HO], F32, tag="ex")
                            nc.scalar.activation(ex[:], mn[:], AF.Exp)
                            rl = wk.tile([128, cfg.HO], F32, tag="rl")
                            nc.vector.tensor_scalar_max(rl[:], h1f[:], 0.0)
                            h2b = wk.tile([128, cfg.HO], BF16, tag="h2b")
                            nc.vector.scalar_tensor_tensor(
                                out=h2b[:], in0=ex[:], scalar=-1.0, in1=rl[:],
                                op0=ALU.add, op1=ALU.add)
                            tp2 = pstr.tile([128, 4, 128], BF16, tag="tp")
                            nc.tensor.transpose(tp2[:, 0, :], h2b[:], ident[:])
                            h2T = wk.tile([128, 128], BF16, tag="h2T")
                            nc.scalar.activation(h2T[:], tp2[:, 0, :], AF.Copy)
                            ps3 = psx.tile([128, cfg.N_CLS + 2], F32, tag="ps3")
                            nc.tensor.matmul(ps3[:], lhsT=h2T[:], rhs=w2_t[:], start=True, stop=True)
                            l2r = wk.tile([128, cfg.L2W], BF16, tag="l2r")
                            nc.scalar.activation(l2r[:, 0:cfg.N_CLS], ps3[:, 0:cfg.N_CLS], AF.Copy)
                            nc.vector.memset(l2r[:, cfg.N_CLS:cfg.N_CLS + 1], 1.0)
                            nc.scalar.activation(l2r[:, cfg.N_CLS + 1:cfg.N_CLS + 2],
                                                 ps3[:, cfg.N_CLS:cfg.N_CLS + 1], AF.Copy)
                            nc.scalar.activation(ad2all[:, b:b + 1],
                                                 ps3[:, cfg.N_CLS + 1:cfg.N_CLS + 2], AF.Copy)
                            nc.sync.dma_start(l2own[b * 128:(b + 1) * 128, 0:cfg.L2W], l2r[:])
                        else:
                            zt2 = scp.tile([128, 1], F32, tag="zt2")
                            nc.vector.tensor_scalar_max(zt2[:], pblk[:, cfg.N_CLS:cfg.N_CLS + 1], 1e-30)
                            rz2 = scp.tile([128, 1], F32, tag="rz2")
                            nc.vector.reciprocal(rz2[:], zt2[:])
                            of = wk.tile([128, cfg.N_CLS], F32, tag="of")
                            nc.scalar.activation(of[:], pblk[:, 0:cfg.N_CLS], AF.Copy,
                                                 scale=rz2[:, 0:1])
                            nc.sync.dma_start(out[b * 128:(b + 1) * 128, :], of[:])

            edge_layer(1)
            # ---------------- P2: AllGather L2 table ----------------------
            nc.gpsimd.collective_compute(
                "AllGather", ALU.bypass,
                replica_groups=[list(range(cfg.n_cores))],
                ins=[l2own[:]], outs=[l2tab[:]])
            hi_elems = (cfg.NG - cfg.SPLIT) * cfg.L2ROW
            CH = 16384
            assert hi_elems % CH == 0, hi_elems
            nc.sync.dma_start(
                l2hi[:, :].flatten().rearrange("(a b) -> a b", b=CH),
                l2tab[:, :].flatten()[cfg.SPLIT * cfg.L2ROW:].rearrange("(a b) -> a b", b=CH))
            edge_layer(2)

    nc.compile()
    if legalize:
        legalize_waits(nc)
    return nc


# ------------------------------------------------------------ numpy ref ---
def numpy_ref(cfg, inputs):
    x = np.asarray(inputs["x"], np.float32)
    W1 = np.asarray(inputs["W1"], np.float32)
    as1 = np.asarray(inputs["att_src1"], np.float32)
    ad1 = np.asarray(inputs["att_dst1"], np.float32)
    W2 = np.asarray(inputs["W2"], np.float32)
    as2 = np.asarray(inputs["att_src2"], np.float32)
    ad2 = np.asarray(inputs["att_dst2"], np.float32)
    ei = np.asarray(inputs["edge_index"])
    N = cfg.N
    loops = np.arange(N, dtype=np.int64)
    src = np.concatenate([ei[0].astype(np.int64), loops])
    dst = np.concatenate([ei[1].astype(np.int64), loops])

    def lrelu(v):
        return np.where(v > 0, v, cfg.neg_slope * v)

    def gat(x, W, att_s, att_d, heads, out_ch):
        h = (x @ W).reshape(N, heads, out_ch)
        a_s = (h * att_s).sum(-1)
        a_d = (h * att_d).sum(-1)
        e = lrelu(a_s[src] + a_d[dst])
        ex = np.exp(e)
        z = np.zeros((N, heads), np.float32)
        np.add.at(z, dst, ex)
        alpha = ex / z[dst]
        msg = h[src] * alpha[:, :, None]
        o = np.zeros((N, heads, out_ch), np.float32)
        np.add.at(o, dst, msg)
        return o.reshape(N, heads * out_ch)

    h = gat(x, W1, as1, ad1, cfg.HEADS, cfg.HID)
    h = np.where(h > 0, h, np.exp(np.minimum(h, 0)) - 1)
    o = gat(h, W2, as2, ad2, 1, cfg.N_CLS)
    return o


def assemble_output(cfg, shared, results):
    full = np.zeros((cfg.N, cfg.N_CLS), np.float32)
    for k in range(cfg.n_cores):
        perm = shared["metas"][k]["perm"]
        o = results[k]["out"]  # [NPCP, 40]
        for b in range(cfg.n_blocks):
            for j in range(128):
                v = perm[b, j]
                if v >= 0:
                    full[k * cfg.NPC + v] = o[b * 128 + j]
    return full


# ================================================================ entry ===
_CACHE = {}

def kernel(**inputs):
    """Full-input GAT kernel: shards across 8 TRN2 NeuronCores internally."""
    cfg = Cfg()  # problem-size defaults
    install_ntff_hook()
    from concourse.bass_utils import run_bass_kernel_spmd
    per_core, shared = prep_host(cfg, inputs)
    key = (cfg.C_lo, cfg.C_hi, cfg.C2_lo, cfg.C2_hi)
    if key not in _CACHE:
        _CACHE[key] = build_kernel(cfg, shared)
    nc = _CACHE[key]
    want_trace = os.environ.get("GAT_TRACE", "1") == "1"
    try:
        res = run_bass_kernel_spmd(nc, per_core, core_ids=list(range(cfg.n_cores)),
                                   trace=want_trace)
    except Exception:
        if not want_trace:
            raise
        res = run_bass_kernel_spmd(nc, per_core, core_ids=list(range(cfg.n_cores)),
                                   trace=False)
    global LAST_EXEC_NS
    LAST_EXEC_NS = res.exec_time_ns
    return assemble_output(cfg, shared, res.results)


LAST_EXEC_NS = None



# revision 7
# speedup vs baseline: 1.4573x; 1.2543x over previous
"""GAT (2 layers: 4 heads x 32, then 1 head x 40) on 8 TRN2 NeuronCores.

Sharding: dst-node sharding across 8 cores. Per core, its NPC nodes are
host-packed into `n_blocks` blocks of <=128 nodes, balanced so that every
block's incoming-edge count (split into lo/hi src streams for int16
dma_gather addressing) fits a FIXED number of 128-edge chunks (C_lo / C_hi)
-- this makes the SPMD program structurally identical across cores while all
per-core irregularity lives in the input data (gather indices, dst-slot ids,
permuted x slices).

Per-edge pipeline (one 128-edge chunk):
  dma_gather slab rows [h1 interleaved with 1.0 cols | a_s] by src;
  D[e,d] = onehot(dst_slot) via DVE tensor_scalar is_equal vs iota;
  D_T via PE transpose; a_d per edge = D_T.T @ a_d_block (PE);
  w = exp(leaky_relu(a_s + a_d)) (DVE add, ACT lrelu, ACT exp+broadcast);
  Msg = G * w_exp (DVE); psum_blk += D.T @ Msg (PE) -- the interleaved 1.0
  columns turn into the softmax normalizers z_h.
Block epilogue: h1 = psum/z, h2 = elu(h1), transpose, @ [W2|ws2|wd2] -> L2
table row [h2p(40) | 1.0 | a_s2] + resident a_d2. AllGather L2 table, run the
same edge pipeline for layer 2, out = psum2/z2.
"""
import sys, os, types, time
sys.path.insert(0, '/opt/trn_rl_repo')
import numpy as np
import ml_dtypes

import concourse.bass as bass
import concourse.bacc as bacc
import concourse.mybir as mybir
import concourse.tile as tile
from concourse.vector_clock import ScopedClock
from concourse.masks import make_identity

BF16 = mybir.dt.bfloat16
F32 = mybir.dt.float32
AF = mybir.ActivationFunctionType
ALU = mybir.AluOpType
NPBF = ml_dtypes.bfloat16

# ---------------------------------------------------------------- compat ---
_MAXW = 1

def _patched_drain_and_barrier(self, tick_clock, wait_clock):
    nc = self.nc
    drain_inst = nc.sync.drain()
    wait_clock.add_sem_waits(drain_inst.ins, ScopedClock({None: tick_clock.global_clock}))
    si = drain_inst.ins.sync_info
    waits = list(si.on_wait or [])
    if len(waits) > _MAXW:
        si.on_wait.clear()
        si.on_wait.extend(waits[:_MAXW])
        rest = waits[_MAXW:]
        for i in range(0, len(rest), _MAXW):
            extra = nc.sync.drain()
            esi = extra.ins.sync_info
            if esi is None:
                extra.ins.sync_info = mybir.SyncInfo(on_wait=[], on_update=[])
                esi = extra.ins.sync_info
            esi.on_wait.extend(rest[i:i + _MAXW])
    nc.all_engine_barrier()
    assert self.sems is not None
    popped = nc._tile_sem_poison_stack.pop()
    assert popped is self._sem_poison
    nc.clear_and_free_semaphores(list(self.sems.allocated().values()))
    nc.all_engine_barrier()

tile.TileContext._drain_and_barrier = _patched_drain_and_barrier


def legalize_waits(nc, maxw=_MAXW):
    n_split = 0
    for fn in nc.m.functions:
        for bb in fn.blocks:
            out = []
            for ins in bb.instructions:
                si = ins.sync_info
                waits = list(si.on_wait) if (si is not None and si.on_wait) else []
                if len(waits) > maxw:
                    extras, keep = waits[:-maxw], waits[-maxw:]
                    for j, w in enumerate(extras):
                        nop = mybir.InstEventSemaphore(name=f"{ins.name}_xw{j}", ins=[], outs=[])
                        nop.engine = ins.engine
                        nop.sync_info = mybir.SyncInfo(on_wait=[w], on_update=[])
                        out.append(nop)
                        n_split += 1
                    si.on_wait.clear()
                    si.on_wait.extend(keep)
                out.append(ins)
            bb.instructions[:] = out
    return n_split


def install_ntff_hook():
    if "antenv.axon_hooks" in sys.modules:
        return
    m = types.ModuleType("antenv.axon_hooks")
    _hook = [None]
    m.set_axon_ntff_profile_hook = lambda h: _hook.__setitem__(0, h)
    m.get_axon_ntff_profile_hook = lambda: _hook[0]
    sys.modules["antenv.axon_hooks"] = m
    import antenv
    antenv.axon_hooks = m
    try:
        from trn_agent_boot.trn_boot import _ntff_profile_via_ctypes
        m.set_axon_ntff_profile_hook(_ntff_profile_via_ctypes('/opt/axon/libaxon_pjrt.so'))
    except Exception:
        pass


# ---------------------------------------------------------------- config ---
class Cfg:
    def __init__(self, N=50000, F_IN=128, HID=32, HEADS=4, N_CLS=40, E=800000,
                 n_cores=8, slab_chunks=24, neg_slope=0.2, split=32768):
        self.N, self.F_IN, self.HID, self.HEADS, self.N_CLS, self.E = N, F_IN, HID, HEADS, N_CLS, E
        self.n_cores = n_cores
        self.SC = slab_chunks                 # chunks per gather slab
        self.neg_slope = neg_slope
        assert N % n_cores == 0
        self.NPC = N // n_cores
        self.n_blocks = (self.NPC + 127) // 128
        self.NPCP = self.n_blocks * 128       # padded nodes per core
        self.HO = HEADS * HID                 # 128
        self.L1C = self.HO + HEADS            # 132 msg cols (features + z cols)
        self.L1W = self.L1C + HEADS           # 136 written cols (plus a_s)
        self.L1ROW = 256                      # bf16 elems per table row (512B)
        self.L2C = N_CLS + 1                  # 41 msg cols (h2p + z col)
        self.L2W = N_CLS + 2                  # 42 written cols (plus a_s2)
        self.L2ROW = 128                      # bf16 per row (256B)
        self.SPLIT = split
        assert split % 128 == 0
        self.NP = ((N + 127) // 128) * 128    # padded node count (dense tiles)
        self.NG = self.n_cores * self.NPCP    # global permuted L2 table rows
        # filled by prep: C_lo, C_hi


# ------------------------------------------------------------ host prep ---
def _pack_blocks(cfg, deg_lo, deg_hi):
    """Greedy balance nodes (local ids 0..NPC-1) into n_blocks blocks of
    <=128 nodes minimizing the max lo/hi edge load. Returns perm [n_blocks,128]
    (local node id or -1)."""
    nb = cfg.n_blocks
    order = np.argsort(-(deg_lo + deg_hi), kind='stable')
    cnt = np.zeros(nb, np.int64)
    lo = np.zeros(nb, np.int64)
    hi = np.zeros(nb, np.int64)
    perm = -np.ones((nb, 128), np.int64)
    for v in order:
        w = deg_lo[v] + deg_hi[v]
        # pick feasible block with min total load
        feas = np.where(cnt < 128)[0]
        b = feas[np.argmin(lo[feas] + hi[feas])]
        perm[b, cnt[b]] = v
        cnt[b] += 1
        lo[b] += deg_lo[v]
        hi[b] += deg_hi[v]
    return perm, lo, hi


def prep_core(cfg, src, dst, core):
    """Per-core packing. Returns dict with perm, per-(block,stream) edge
    arrays (src-sorted), and counts."""
    base = core * cfg.NPC
    m = (dst >= base) & (dst < base + cfg.NPC)
    s, d = src[m], dst[m] - base
    deg_lo = np.bincount(d[s < cfg.SPLIT], minlength=cfg.NPC)
    deg_hi = np.bincount(d[s >= cfg.SPLIT], minlength=cfg.NPC)
    perm, lo_sum, hi_sum = _pack_blocks(cfg, deg_lo, deg_hi)
    # node -> (block, slot)
    blk_of = -np.ones(cfg.NPC, np.int64)
    slot_of = -np.ones(cfg.NPC, np.int64)
    for b in range(cfg.n_blocks):
        for j in range(128):
            v = perm[b, j]
            if v >= 0:
                blk_of[v] = b
                slot_of[v] = j
    eb = blk_of[d]
    es = slot_of[d]
    per = {}
    for b in range(cfg.n_blocks):
        mb = eb == b
        sb, slb = s[mb], es[mb]
        for st in (0, 1):
            ms = (sb < cfg.SPLIT) if st == 0 else (sb >= cfg.SPLIT)
            ss, sl = sb[ms], slb[ms]
            o = np.argsort(ss, kind='stable')
            per[(b, st)] = (ss[o], sl[o])
    return {"perm": perm, "per": per,
            "c_lo": int(np.max(np.ceil(lo_sum / 128))) if len(lo_sum) else 0,
            "c_hi": int(np.max(np.ceil(hi_sum / 128))) if len(hi_sum) else 0}


def build_streams(cfg, pc, l2_bases):
    """Given per-core packing pc and fixed C_lo/C_hi, build the gather idx
    arrays and dst-slot arrays for L1 (raw node ids) and L2 (permuted global
    ids via l2_bases[node] = permuted row)."""
    C = {0: cfg.C_lo, 1: cfg.C_hi}
    out = {}
    for layer, xl in (("l1", None), ("l2", l2_bases)):
        for st in (0, 1):
            nchunks = cfg.n_blocks * C[st]
            idx = np.zeros((nchunks, 128), np.int64)
            dl = np.full((nchunks, 128), 255, np.int64)
            ci = 0
            for b in range(cfg.n_blocks):
                ss, sl = pc["per"][(b, st)]
                if layer == "l2":
                    g = xl[ss]                       # permuted global rows
                    g = np.sort(g)                    # locality; slot must follow!
                    # NOTE: need slots aligned with sorted order:
                    o = np.argsort(xl[ss], kind='stable')
                    g, sl2 = xl[ss][o], sl[o]
                    base_off = cfg.SPLIT if st == 1 else 0
                    # re-split: permuted ids may cross SPLIT differently!
                    # handled by caller building separate l2 streams.
                    v = g - base_off
                    svals, slvals = v, sl2
                else:
                    svals = ss - (cfg.SPLIT if st == 1 else 0)
                    slvals = sl
                n = len(svals)
                cap = C[st] * 128
                assert n <= cap, (n, cap)
                a = np.zeros(cap, np.int64)
                a[:n] = svals
                dd = np.full(cap, 255, np.int64)
                dd[:n] = slvals
                for c in range(C[st]):
                    idx[ci] = a[c * 128:(c + 1) * 128]
                    dl[ci] = dd[c * 128:(c + 1) * 128]
                    ci += 1
            out[(layer, st)] = (idx, dl)
    return out


def prep_host(cfg, inputs):
    x = np.asarray(inputs["x"], np.float32)
    W1 = np.asarray(inputs["W1"], np.float32)
    as1 = np.asarray(inputs["att_src1"], np.float32)
    ad1 = np.asarray(inputs["att_dst1"], np.float32)
    W2 = np.asarray(inputs["W2"], np.float32)
    as2 = np.asarray(inputs["att_src2"], np.float32)
    ad2 = np.asarray(inputs["att_dst2"], np.float32)
    ei = np.asarray(inputs["edge_index"])
    loops = np.arange(cfg.N, dtype=np.int64)
    src = np.concatenate([ei[0].astype(np.int64), loops])
    dst = np.concatenate([ei[1].astype(np.int64), loops])

    H, HID = cfg.HEADS, cfg.HID
    # W1il: interleaved feature cols with zero cols at the z positions
    W1r = W1.reshape(cfg.F_IN, H, HID)
    W1il = np.zeros((cfg.F_IN, cfg.L1C), np.float32)
    for h in range(H):
        W1il[:, 33 * h:33 * h + HID] = W1r[:, h]
    Ws1 = (W1r * as1[None]).sum(-1)
    Wd1 = (W1r * ad1[None]).sum(-1)
    W1ext = np.concatenate([W1il, Ws1, Wd1], 1).astype(NPBF)   # [128, 140]
    ws2 = (W2 * as2).sum(1, keepdims=True)
    wd2 = (W2 * ad2).sum(1, keepdims=True)
    W2ext = np.concatenate([W2, ws2, wd2], 1).astype(NPBF)     # [128, 42]

    xT = np.zeros((cfg.F_IN, cfg.NP), NPBF)
    xT[:, :cfg.N] = np.ascontiguousarray(x.T).astype(NPBF)
    iota = np.broadcast_to(np.arange(128, dtype=np.float32), (128, 128)).astype(NPBF)

    # pass 1: per-core packing, find global C_lo/C_hi
    pcs = [prep_core(cfg, src, dst, k) for k in range(cfg.n_cores)]
    cfg.C_lo = max(p["c_lo"] for p in pcs)
    cfg.C_hi = max(p["c_hi"] for p in pcs)

    # L2 permuted global row of node v
    l2_bases = np.zeros(cfg.N, np.int64)
    for k in range(cfg.n_cores):
        perm = pcs[k]["perm"]
        for b in range(cfg.n_blocks):
            for j in range(128):
                v = perm[b, j]
                if v >= 0:
                    l2_bases[k * cfg.NPC + v] = k * cfg.NPCP + b * 128 + j

    # L2 streams must be re-split by PERMUTED id (lo: <SPLIT, hi: >=SPLIT).
    # Simplest correct approach: recompute per-core per-block streams in the
    # permuted-id space with its own balance constraint... but the block
    # structure (C_lo/C_hi chunks) is shared with L1. Instead: keep the SAME
    # chunk structure (same edge->chunk assignment as L1) and allow l2 idx
    # to go out of int16 range? No. Resolution: build L2 edge streams
    # independently (re-sorted by permuted src id and re-split lo/hi), with
    # their own fixed chunk counts C2_lo/C2_hi.
    per2 = []
    c2lo = c2hi = 0
    for k in range(cfg.n_cores):
        base = k * cfg.NPC
        m = (dst >= base) & (dst < base + cfg.NPC)
        s0, d0 = src[m], dst[m] - base
        perm = pcs[k]["perm"]
        blk_of = -np.ones(cfg.NPC, np.int64)
        slot_of = -np.ones(cfg.NPC, np.int64)
        for b in range(cfg.n_blocks):
            for j in range(128):
                v = perm[b, j]
                if v >= 0:
                    blk_of[v] = b
                    slot_of[v] = j
        g = l2_bases[s0]                       # permuted global src row
        eb, es = blk_of[d0], slot_of[d0]
        per = {}
        for b in range(cfg.n_blocks):
            mb = eb == b
            gb, sb = g[mb], es[mb]
            for st in (0, 1):
                ms = (gb < cfg.SPLIT) if st == 0 else (gb >= cfg.SPLIT)
                gg, sl = gb[ms], sb[ms]
                o = np.argsort(gg, kind='stable')
                per[(b, st)] = (gg[o], sl[o])
                n = len(gg)
                if st == 0:
                    c2lo = max(c2lo, int(np.ceil(n / 128)))
                else:
                    c2hi = max(c2hi, int(np.ceil(n / 128)))
        per2.append(per)
    cfg.C2_lo, cfg.C2_hi = c2lo, c2hi

    def pack_stream(per, b_range, st, C, base_off):
        nchunks = cfg.n_blocks * C
        idx = np.zeros((nchunks, 128), np.int64)
        dl = np.full((nchunks, 128), 255, np.int64)
        ci = 0
        for b in b_range:
            ss, sl = per[(b, st)]
            n = len(ss)
            cap = C * 128
            assert n <= cap
            a = np.zeros(cap, np.int64)
            a[:n] = ss - base_off
            dd = np.full(cap, 255, np.int64)
            dd[:n] = sl
            for c in range(C):
                idx[ci] = a[c * 128:(c + 1) * 128]
                dl[ci] = dd[c * 128:(c + 1) * 128]
                ci += 1
        return idx, dl

    def to_gather_layout(cfg, idx_chunks, dl_chunks, SC):
        """idx_chunks [nch, 128] -> slabs of SC chunks:
        idx16 [nslab, 128, SC*8] (i at partition i%16, free i//16, 8x stripes)
        dl bf16 [nslab, 128, SC]."""
        nch = idx_chunks.shape[0]
        nslab = (nch + SC - 1) // SC
        idx16 = np.zeros((nslab, 128, SC * 8), np.int16)
        dlb = np.full((nslab, 128, SC), 255.0, np.float32)
        sizes = []
        for si in range(nslab):
            c0, c1 = si * SC, min((si + 1) * SC, nch)
            n = (c1 - c0) * 128
            flat = idx_chunks[c0:c1].reshape(-1)   # chunk-major, i = c*128+p
            w = flat.astype(np.int16).reshape(-1, 16).T  # [16, n/16]
            idx16[si, :, :w.shape[1]] = np.tile(w, (8, 1))
            dlb[si, :, :c1 - c0] = dl_chunks[c0:c1].T.astype(np.float32)
            sizes.append(n)
        return idx16, dlb, sizes

    per_core, metas = [], []
    for k in range(cfg.n_cores):
        pc, p2 = pcs[k], per2[k]
        im = {"xT": np.asarray(xT), "W1ext": np.asarray(W1ext),
              "W2ext": np.asarray(W2ext), "iota": np.asarray(iota)}
        # permuted own x slice (padded slots -> zero cols)
        xo = np.zeros((cfg.F_IN, cfg.NPCP), NPBF)
        perm = pc["perm"]
        for b in range(cfg.n_blocks):
            for j in range(128):
                v = perm[b, j]
                if v >= 0:
                    xo[:, b * 128 + j] = xT[:, k * cfg.NPC + v]
        im["xT_own"] = xo
        meta = {"perm": perm}
        for tag, per, Clo, Chi, split in (
                ("a", pc["per"], cfg.C_lo, cfg.C_hi, cfg.SPLIT),
                ("b", p2, cfg.C2_lo, cfg.C2_hi, cfg.SPLIT)):
            for st, C in ((0, Clo), (1, Chi)):
                idxc, dlc = pack_stream(per, range(cfg.n_blocks), st, C, split if st else 0)
                i16, dlb, sizes = to_gather_layout(cfg, idxc, dlc, cfg.SC)
                im[f"idx_{tag}{st}"] = i16
                im[f"dl_{tag}{st}"] = np.asarray(dlb)
                meta[f"sizes_{tag}{st}"] = sizes
        per_core.append(im)
        metas.append(meta)
    shared = {"metas": metas, "l2_bases": l2_bases, "pcs": pcs}
    return per_core, shared


# ------------------------------------------------------------ device build ---
def build_kernel(cfg, shared, legalize=True):
    SC = cfg.SC
    sizes_a0 = shared["metas"][0]["sizes_a0"]
    sizes_a1 = shared["metas"][0]["sizes_a1"]
    sizes_b0 = shared["metas"][0]["sizes_b0"]
    sizes_b1 = shared["metas"][0]["sizes_b1"]
    ns_a0, ns_a1 = len(sizes_a0), len(sizes_a1)
    ns_b0, ns_b1 = len(sizes_b0), len(sizes_b1)
    NT = cfg.NP // 128
    H = cfg.HEADS

    nc = bacc.Bacc("TRN2", target_bir_lowering=False, num_devices=cfg.n_cores)

    xT = nc.dram_tensor("xT", [cfg.F_IN, cfg.NP], BF16, kind="ExternalInput")
    xT_own = nc.dram_tensor("xT_own", [cfg.F_IN, cfg.NPCP], BF16, kind="ExternalInput")
    W1ext = nc.dram_tensor("W1ext", [cfg.F_IN, cfg.L1C + 2 * H], BF16, kind="ExternalInput")
    W2ext = nc.dram_tensor("W2ext", [cfg.HO, cfg.N_CLS + 2], BF16, kind="ExternalInput")
    iota_in = nc.dram_tensor("iota", [128, 128], BF16, kind="ExternalInput")
    gin = {}
    for tag, ns in (("a0", ns_a0), ("a1", ns_a1), ("b0", ns_b0), ("b1", ns_b1)):
        gin[f"idx_{tag}"] = nc.dram_tensor(f"idx_{tag}", [max(ns, 1), 128, SC * 8], mybir.dt.int16, kind="ExternalInput")
        gin[f"dl_{tag}"] = nc.dram_tensor(f"dl_{tag}", [max(ns, 1), 128, SC], F32, kind="ExternalInput")
    out = nc.dram_tensor("out", [cfg.NPCP, cfg.N_CLS], F32, kind="ExternalOutput")

    l1lo = nc.dram_tensor("l1lo", [cfg.SPLIT, cfg.L1ROW], BF16)
    l1hi = nc.dram_tensor("l1hi", [cfg.NP - cfg.SPLIT, cfg.L1ROW], BF16)
    l2own = nc.dram_tensor("l2own", [cfg.NPCP, cfg.L2ROW], BF16)
    l2tab = nc.dram_tensor("l2tab", [cfg.NG, cfg.L2ROW], BF16, addr_space="Shared")
    l2hi = nc.dram_tensor("l2hi", [cfg.NG - cfg.SPLIT, cfg.L2ROW], BF16)

    with tile.TileContext(nc) as tc:
        regcache = {}
        def reg(v):
            if v not in regcache:
                regcache[v] = nc.gpsimd.to_reg(v)
            return regcache[v]

        with tc.tile_pool(name="consts", bufs=1) as consts:
            iota_t = consts.tile([128, 128], BF16)
            nc.sync.dma_start(iota_t[:], iota_in[:])
            ident = consts.tile([128, 128], BF16)
            make_identity(nc, ident[:])
            w1_t = consts.tile([cfg.F_IN, cfg.L1C + 2 * H], BF16)
            nc.sync.dma_start(w1_t[:], W1ext[:])
            w2_t = consts.tile([cfg.HO, cfg.N_CLS + 2], BF16)
            nc.sync.dma_start(w2_t[:], W2ext[:])
            adall = consts.tile([128, cfg.n_blocks * H], BF16)
            ad2all = consts.tile([128, cfg.n_blocks], BF16)
            # iota replicated 12x along a middle dim for group-batched
            # one-hot builds (is_equal against a broadcast dst-slot vector)
            iota12 = consts.tile([128, 12, 128], BF16)
            for j in range(12):
                nc.vector.tensor_scalar(iota12[:, j, :], iota_t[:], 0.0, None, ALU.add)

            # ---------------- P0: dense L1 table over all nodes ----------
            # 3 node-tiles per group: 3 matmuls into one PSUM bank, one ACT
            # copy, one strided memset, one (or two, at the lo/hi boundary)
            # table DMA -- cuts ACT + DMA instruction counts ~3x.
            GT = 3
            tl = cfg.SPLIT // 128
            with (
                tc.tile_pool(name="p0", bufs=4) as p0pool,
                tc.tile_pool(name="p0ps", bufs=2, space="PSUM") as p0ps,
            ):
                for gi in range((NT + GT - 1) // GT):
                    t0 = gi * GT
                    nh = min(GT, NT - t0)
                    xt = p0pool.tile([cfg.F_IN, GT * 128], BF16, tag="xt")
                    eng_in = nc.scalar if (gi % 2) else nc.sync
                    eng_in.dma_start(xt[:, 0:nh * 128], xT[:, t0 * 128:(t0 + nh) * 128])
                    ps = p0ps.tile([128, GT, cfg.L1W], F32, tag="p0ps")
                    for j in range(nh):
                        nc.tensor.matmul(ps[:, j, :], lhsT=xt[:, j * 128:(j + 1) * 128],
                                         rhs=w1_t[:, 0:cfg.L1W], start=True, stop=True)
                    ev = p0pool.tile([128, GT, cfg.L1W], BF16, tag="ev")
                    nc.scalar.activation(ev[:, 0:nh, :], ps[:, 0:nh, :], AF.Copy)
                    nc.vector.memset(ev[:, 0:nh, 32:cfg.L1C:33], 1.0)
                    weng = nc.sync if (gi % 2) else nc.scalar
                    jlo = max(0, min(nh, tl - t0))
                    if jlo > 0:
                        weng.dma_start(
                            l1lo[t0 * 128:(t0 + jlo) * 128, 0:cfg.L1W]
                            .rearrange("(j p) c -> p j c", p=128),
                            ev[:, 0:jlo, :])
                    if jlo < nh:
                        th0 = t0 + jlo - tl
                        weng.dma_start(
                            l1hi[th0 * 128:(th0 + nh - jlo) * 128, 0:cfg.L1W]
                            .rearrange("(j p) c -> p j c", p=128),
                            ev[:, jlo:nh, :])
                # own-range a_d (permuted block layout), 4 blocks per DMA/copy
                for b0 in range(0, cfg.n_blocks, 4):
                    nb4 = min(4, cfg.n_blocks - b0)
                    xo = p0pool.tile([cfg.F_IN, 512], BF16, tag="xt")
                    nc.sync.dma_start(xo[:, 0:nb4 * 128],
                                      xT_own[:, b0 * 128:(b0 + nb4) * 128])
                    ps = p0ps.tile([128, 4, H], F32, tag="p0psb")
                    for j in range(nb4):
                        nc.tensor.matmul(ps[:, j, :], lhsT=xo[:, j * 128:(j + 1) * 128],
                                         rhs=w1_t[:, cfg.L1C + H:cfg.L1C + 2 * H],
                                         start=True, stop=True)
                    nc.scalar.activation(adall[:, H * b0:H * (b0 + nb4)], ps[:, 0:nb4, :], AF.Copy)

            # ---------------- edge-layer template -------------------------
            def edge_layer(layer):
                if layer == 1:
                    tab_lo, tab_hi, tag_lo, tag_hi = l1lo, l1hi, "a0", "a1"
                    C_lo, C_hi = cfg.C_lo, cfg.C_hi
                    sizes = {0: sizes_a0, 1: sizes_a1}
                    ROW, MC = cfg.L1ROW, cfg.L1C
                else:
                    tab_lo, tab_hi, tag_lo, tag_hi = l2tab, l2hi, "b0", "b1"
                    C_lo, C_hi = cfg.C2_lo, cfg.C2_hi
                    sizes = {0: sizes_b0, 1: sizes_b1}
                    ROW, MC = cfg.L2ROW, cfg.L2C
                NH = H if layer == 1 else 1

                with (
                    tc.tile_pool(name=f"gsl{layer}", bufs=3) as gpool_lo,
                    tc.tile_pool(name=f"gsh{layer}", bufs=3) as gpool_hi,
                    tc.tile_pool(name=f"dl{layer}", bufs=3) as dlpool,
                    tc.tile_pool(name=f"ix{layer}", bufs=3) as ixpool,
                    tc.tile_pool(name=f"wk{layer}", bufs=4) as wk,
                    tc.tile_pool(name=f"dp{layer}", bufs=3) as dp,
                    tc.tile_pool(name=f"dtp{layer}", bufs=4) as dtp,
                    tc.tile_pool(name=f"sc{layer}", bufs=4) as scp,
                    tc.tile_pool(name=f"ps{layer}", bufs=2, space="PSUM") as psacc,
                    tc.tile_pool(name=f"pst{layer}", bufs=2, space="PSUM") as pstr,
                    tc.tile_pool(name=f"psx{layer}", bufs=2, space="PSUM") as psx,
                ):
                    cur = {0: -1, 1: -1}
                    slabs = {0: None, 1: None}
                    dls = {0: None, 1: None}

                    def ensure_slab(st, si):
                        if cur[st] == si:
                            return
                        cur[st] = si
                        pool = gpool_lo if st == 0 else gpool_hi
                        tname = tag_lo if st == 0 else tag_hi
                        it = ixpool.tile([128, SC * 8], mybir.dt.int16, tag=f"it{st}")
                        nc.sync.dma_start(it[:], gin[f"idx_{tname}"][si])
                        dlt = dlpool.tile([128, SC], F32, tag=f"dl{st}")
                        nc.sync.dma_start(dlt[:], gin[f"dl_{tname}"][si])
                        g = pool.tile([128, SC, ROW], BF16, tag=f"g{st}")
                        n = sizes[st][si]
                        tabx = tab_lo if st == 0 else tab_hi
                        nc.gpsimd.dma_gather(
                            out_ap=g[:, 0:max(n // 128, 1), :],
                            in_ap=tabx[:, :],
                            idxs_ap=it[:, 0:max(n // 16, 1)],
                            num_idxs=n, num_idxs_reg=reg(n),
                            elem_size=ROW, single_packet=False)
                        slabs[st] = g
                        dls[st] = dlt

                    for b in range(cfg.n_blocks):
                        pblk = psacc.tile([128, MC], F32, tag="acc")
                        first = True
                        # chunk list: (stream, global chunk id)
                        chs = [(0, b * C_lo + c) for c in range(C_lo)] + \
                              [(1, b * C_hi + c) for c in range(C_hi)]
                        # group for score batching: consecutive chunks in same slab
                        groups = []
                        gcur = None
                        for st, gc in chs:
                            si, col = gc // SC, gc % SC
                            if gcur and gcur[0] == st and gcur[1] == si and len(gcur[2]) < 12:
                                gcur[2].append(col)
                            else:
                                gcur = [st, si, [col]]
                                groups.append(gcur)
                        nch = len(chs)
                        ci_total = 0
                        for st, si, cols in groups:
                            ensure_slab(st, si)
                            g, dlt = slabs[st], dls[st]
                            ng = len(cols)
                            c0 = cols[0]
                            # one batched one-hot build for the whole group:
                            # D[e, j, d] = (iota[d] == dl[e, c0+j])
                            Dg = dp.tile([128, 12, 128], BF16, tag="D")
                            nc.vector.tensor_tensor(
                                out=Dg[:, 0:ng, :], in0=iota12[:, 0:ng, :],
                                in1=dlt[:, c0:c0 + ng].to_broadcast([128, ng, 128]),
                                op=ALU.is_equal)
                            # transposes in sub-batches of 4, one PSUM->SBUF copy each
                            adx = psx.tile([128, 12 * NH], F32, tag="adx")
                            for q0 in range(0, ng, 4):
                                qn = min(4, ng - q0)
                                tp4 = pstr.tile([128, 4, 128], BF16, tag="tp")
                                for q in range(qn):
                                    nc.tensor.transpose(tp4[:, q, :], Dg[:, q0 + q, :], ident[:])
                                DT4 = dtp.tile([128, 4, 128], BF16, tag="DT")
                                nc.scalar.activation(DT4[:, 0:qn, :], tp4[:, 0:qn, :], AF.Copy)
                                for q in range(qn):
                                    nc.tensor.matmul(
                                        adx[:, (q0 + q) * NH:(q0 + q + 1) * NH],
                                        lhsT=DT4[:, q, :],
                                        rhs=(adall[:, H * b:H * b + H] if layer == 1
                                             else ad2all[:, b:b + 1]),
                                        start=True, stop=True)
                            # batched scores for the group
                            sc_t = scp.tile([128, 12 * NH], F32, tag="sc")
                            if layer == 1:
                                asrc = g[:, c0:c0 + ng, cfg.L1C:cfg.L1C + H]
                            else:
                                asrc = g[:, c0:c0 + ng, cfg.N_CLS + 1:cfg.N_CLS + 2]
                            nc.vector.tensor_tensor(
                                out=sc_t[:, 0:ng * NH], in0=adx[:, 0:ng * NH],
                                in1=asrc, op=ALU.add)
                            nc.vector.scalar_tensor_tensor(
                                out=sc_t[:, 0:ng * NH], in0=sc_t[:, 0:ng * NH],
                                scalar=cfg.neg_slope, in1=sc_t[:, 0:ng * NH],
                                op0=ALU.mult, op1=ALU.max)
                            msg = wk.tile([128, 12, MC], BF16, tag="msg")
                            if layer == 1:
                                # one broadcast-exp for the whole group, then one
                                # batched per-edge multiply over [128, ng, 132]
                                wex = wk.tile([128, 12, cfg.L1C], BF16, tag="wex")
                                nc.scalar.activation(
                                    wex[:, 0:ng, :].rearrange("p g (h q) -> p (g h) q", q=33),
                                    sc_t[:, 0:ng * H].to_broadcast([128, ng * H, 33]),
                                    AF.Exp)
                                nc.vector.tensor_tensor(
                                    out=msg[:, 0:ng, :], in0=g[:, c0:c0 + ng, 0:MC],
                                    in1=wex[:, 0:ng, :], op=ALU.mult)
                            else:
                                # per-partition scale path on ACT (frees DVE)
                                w2s = scp.tile([128, 12], F32, tag="w2s")
                                nc.scalar.activation(w2s[:, 0:ng], sc_t[:, 0:ng], AF.Exp)
                                for j in range(ng):
                                    nc.scalar.activation(msg[:, j, :], g[:, c0 + j, 0:MC],
                                                         AF.Copy, scale=w2s[:, j:j + 1])
                            for j, col in enumerate(cols):
                                ci_total += 1
                                nc.tensor.matmul(pblk[:], lhsT=Dg[:, j, :], rhs=msg[:, j, :],
                                                 start=first, stop=(ci_total == nch))
                                first = False
                        # -------- block epilogue --------
                        if layer == 1:
                            zt = scp.tile([128, H], F32, tag="zt")
                            nc.vector.tensor_scalar_max(zt[:], pblk[:, 32:cfg.L1C:33], 1e-30)
                            rz = scp.tile([128, H], F32, tag="rz")
                            nc.vector.reciprocal(rz[:], zt[:])
                            h1f = wk.tile([128, cfg.HO], F32, tag="h1f")
                            for h in range(H):
                                nc.scalar.activation(
                                    h1f[:, 32 * h:32 * h + 32],
                                    pblk[:, 33 * h:33 * h + 32],
                                    AF.Copy, scale=rz[:, h:h + 1])
                            mn = wk.tile([128, cfg.HO], F32, tag="mn")
                            nc.vector.tensor_scalar_min(mn[:], h1f[:], 0.0)
                            ex = wk.tile([128, cfg.HO], F32, tag="ex")
                            nc.scalar.activation(ex[:], mn[:], AF.Exp)
                            rl = wk.tile([128, cfg.HO], F32, tag="rl")
                            nc.vector.tensor_scalar_max(rl[:], h1f[:], 0.0)
                            h2b = wk.tile([128, cfg.HO], BF16, tag="h2b")
                            nc.vector.scalar_tensor_tensor(
                                out=h2b[:], in0=ex[:], scalar=-1.0, in1=rl[:],
                                op0=ALU.add, op1=ALU.add)
                            tp2 = pstr.tile([128, 4, 128], BF16, tag="tp")
                            nc.tensor.transpose(tp2[:, 0, :], h2b[:], ident[:])
                            h2T = wk.tile([128, 128], BF16, tag="h2T")
                            nc.scalar.activation(h2T[:], tp2[:, 0, :], AF.Copy)
                            ps3 = psx.tile([128, cfg.N_CLS + 2], F32, tag="ps3")
                            nc.tensor.matmul(ps3[:], lhsT=h2T[:], rhs=w2_t[:], start=True, stop=True)
                            l2r = wk.tile([128, cfg.L2W], BF16, tag="l2r")
                            nc.scalar.activation(l2r[:, 0:cfg.N_CLS], ps3[:, 0:cfg.N_CLS], AF.Copy)
                            nc.vector.memset(l2r[:, cfg.N_CLS:cfg.N_CLS + 1], 1.0)
                            nc.scalar.activation(l2r[:, cfg.N_CLS + 1:cfg.N_CLS + 2],
                                                 ps3[:, cfg.N_CLS:cfg.N_CLS + 1], AF.Copy)
                            nc.scalar.activation(ad2all[:, b:b + 1],
                                                 ps3[:, cfg.N_CLS + 1:cfg.N_CLS + 2], AF.Copy)
                            nc.sync.dma_start(l2own[b * 128:(b + 1) * 128, 0:cfg.L2W], l2r[:])
                        else:
                            zt2 = scp.tile([128, 1], F32, tag="zt2")
                            nc.vector.tensor_scalar_max(zt2[:], pblk[:, cfg.N_CLS:cfg.N_CLS + 1], 1e-30)
                            rz2 = scp.tile([128, 1], F32, tag="rz2")
                            nc.vector.reciprocal(rz2[:], zt2[:])
                            of = wk.tile([128, cfg.N_CLS], F32, tag="of")
                            nc.scalar.activation(of[:], pblk[:, 0:cfg.N_CLS], AF.Copy,
                                                 scale=rz2[:, 0:1])
                            nc.sync.dma_start(out[b * 128:(b + 1) * 128, :], of[:])

            edge_layer(1)
            # ---------------- P2: AllGather L2 table ----------------------
            nc.gpsimd.collective_compute(
                "AllGather", ALU.bypass,
                replica_groups=[list(range(cfg.n_cores))],
                ins=[l2own[:]], outs=[l2tab[:]])
            hi_elems = (cfg.NG - cfg.SPLIT) * cfg.L2ROW
            CH = 16384
            assert hi_elems % CH == 0, hi_elems
            nc.sync.dma_start(
                l2hi[:, :].flatten().rearrange("(a b) -> a b", b=CH),
                l2tab[:, :].flatten()[cfg.SPLIT * cfg.L2ROW:].rearrange("(a b) -> a b", b=CH))
            edge_layer(2)

    nc.compile()
    if legalize:
        legalize_waits(nc)
    return nc


# ------------------------------------------------------------ numpy ref ---
def numpy_ref(cfg, inputs):
    x = np.asarray(inputs["x"], np.float32)
    W1 = np.asarray(inputs["W1"], np.float32)
    as1 = np.asarray(inputs["att_src1"], np.float32)
    ad1 = np.asarray(inputs["att_dst1"], np.float32)
    W2 = np.asarray(inputs["W2"], np.float32)
    as2 = np.asarray(inputs["att_src2"], np.float32)
    ad2 = np.asarray(inputs["att_dst2"], np.float32)
    ei = np.asarray(inputs["edge_index"])
    N = cfg.N
    loops = np.arange(N, dtype=np.int64)
    src = np.concatenate([ei[0].astype(np.int64), loops])
    dst = np.concatenate([ei[1].astype(np.int64), loops])

    def lrelu(v):
        return np.where(v > 0, v, cfg.neg_slope * v)

    def gat(x, W, att_s, att_d, heads, out_ch):
        h = (x @ W).reshape(N, heads, out_ch)
        a_s = (h * att_s).sum(-1)
        a_d = (h * att_d).sum(-1)
        e = lrelu(a_s[src] + a_d[dst])
        ex = np.exp(e)
        z = np.zeros((N, heads), np.float32)
        np.add.at(z, dst, ex)
        alpha = ex / z[dst]
        msg = h[src] * alpha[:, :, None]
        o = np.zeros((N, heads, out_ch), np.float32)
        np.add.at(o, dst, msg)
        return o.reshape(N, heads * out_ch)

    h = gat(x, W1, as1, ad1, cfg.HEADS, cfg.HID)
    h = np.where(h > 0, h, np.exp(np.minimum(h, 0)) - 1)
    o = gat(h, W2, as2, ad2, 1, cfg.N_CLS)
    return o


def assemble_output(cfg, shared, results):
    full = np.zeros((cfg.N, cfg.N_CLS), np.float32)
    for k in range(cfg.n_cores):
        perm = shared["metas"][k]["perm"]
        o = results[k]["out"]  # [NPCP, 40]
        for b in range(cfg.n_blocks):
            for j in range(128):
                v = perm[b, j]
                if v >= 0:
                    full[k * cfg.NPC + v] = o[b * 128 + j]
    return full


# ================================================================ entry ===
_CACHE = {}

def kernel(**inputs):
    """Full-input GAT kernel: shards across 8 TRN2 NeuronCores internally."""
    cfg = Cfg()  # problem-size defaults
    install_ntff_hook()
    from concourse.bass_utils import run_bass_kernel_spmd
    per_core, shared = prep_host(cfg, inputs)
    key = (cfg.C_lo, cfg.C_hi, cfg.C2_lo, cfg.C2_hi)
    if key not in _CACHE:
        _CACHE[key] = build_kernel(cfg, shared)
    nc = _CACHE[key]
    want_trace = os.environ.get("GAT_TRACE", "1") == "1"
    try:
        res = run_bass_kernel_spmd(nc, per_core, core_ids=list(range(cfg.n_cores)),
                                   trace=want_trace)
    except Exception:
        if not want_trace:
            raise
        res = run_bass_kernel_spmd(nc, per_core, core_ids=list(range(cfg.n_cores)),
                                   trace=False)
    global LAST_EXEC_NS
    LAST_EXEC_NS = res.exec_time_ns
    return assemble_output(cfg, shared, res.results)


LAST_EXEC_NS = None

